# revision 38
# baseline (speedup 1.0000x reference)
"""Trainium2 Bass kernel for temporal-decay causal multi-head attention.

Problem: nn_MultiHeadAttention_9053791060340
  B=4, S=2048, DM=512, H=8, HD=64.
  out = softmax((Q K^T / sqrt(HD)) * exp(-rate*|t_i - t_j|) with causal mask) V,
  then out-projection.

Sharding: 8 cores = 4 batches x 2 head-groups (4 heads each). Each core
computes a partial out-projection [S, DM] for its head group; the host sums
the two partials per batch and adds the output bias.

Device algorithm (per core); matmul inputs in bf16 with fp32 PSUM
accumulation by default (set KERNEL_F32=1 for all-fp32, ~2.2x slower,
error ~1e-6 instead of ~3e-3 scale-relative):
  - scores are computed TRANSPOSED (S^T[k, q] = k . q), so softmax-free-axis
    tricks are unnecessary: we use a no-max softmax (scores here are bounded,
    |v| <~ 64, so exp never overflows in fp32), and the denominator comes for
    free from a ones-column appended to V (PV matmul directly accumulates
    ctx^T[hd, q] plus the row of denominators).
  - temporal decay factorizes on sorted days: exp(-r(t_i - t_k)) = a_i * b_k
    with a per-q-group reference t0 to keep the factors in fp32 range.
    a (and the 1/sqrt(HD) scale) is folded into q^T once; b is folded into a
    per-q-group scaled copy of k^T.
  - pairs far enough apart (rate*dist > ~26) have s*decay so small that
    exp(s*decay) == 1.0f exactly (reference behaves identically), so all
    k-chunks entirely below the cutoff collapse into a rank-1 update
    ctx^T += PrefVsum x ones, with PrefVsum precomputed on the host.
  - causal masking needs work only in the diagonal 128x128 band of each
    q-group: an additive -1e30 tril tile is applied to S^T in PSUM before the
    exp. Blocks fully above the diagonal are skipped by restricting the
    streamed q-range.
"""

import os

import ml_dtypes
import numpy as np

import concourse.bass as bass
import concourse.tile as tile
from concourse import bacc
from concourse import mybir
from concourse.bass_utils import run_bass_kernel_spmd
from concourse.masks import make_identity

F32 = mybir.dt.float32

B, S, DM, H = 4, 2048, 512, 8
HD = DM // H          # 64
NCORES = 8
NHG = 2               # head groups == cores per batch
NH = H // NHG         # heads per core
HGD = NH * HD         # 256 output dims per core
QG = 512              # q-group width
NQG = S // QG         # 4
KC = 128              # k chunk (partition dim of S^T)
NKC = S // KC         # 16
P = 128
NEG = -1.0e30

_cache: dict = {}


# --------------------------------------------------------------------------
# device program
# --------------------------------------------------------------------------

def _build_fast(kc_lo: tuple, wmax: int, with_bqk: bool, with_bv: bool,
                use_bf16: bool = True):
    """Build the SPMD Bass program.

    kc_lo[qg] = first near k-chunk per q-group (static across cores; chunks
    below it are covered by the prefix-sum rank-1 update). wmax = max
    near-window width in elements for the scaled-k tile. use_bf16 casts all
    matmul inputs to bf16 (fp32 PSUM accumulation): the PE streams 1 col/cyc
    for bf16 vs 2 for fp32, halving matmul time.

    The q-group loop is software-pipelined: prep(qg+1) (b-vector DMA +
    scaled-k) is emitted before tail(qg) (reciprocal, divide, out-project)
    so the PE always has next-group score matmuls available while the
    denominator pipeline of the previous group drains.
    """
    nc = bacc.Bacc()
    MDT = mybir.dt.bfloat16 if use_bf16 else F32

    xT_d = nc.declare_dram_parameter("xT", [DM, S], MDT, False)
    wq_d = nc.declare_dram_parameter("wqT", [DM, HGD], MDT, False)
    wk_d = nc.declare_dram_parameter("wkT", [DM, HGD], MDT, False)
    wv_d = nc.declare_dram_parameter("wvT", [DM, HGD], MDT, False)
    wo_d = nc.declare_dram_parameter("woT", [HGD, DM], MDT, False)
    av_d = nc.declare_dram_parameter("avec", [1, S], F32, False)
    bv_d = nc.declare_dram_parameter("bvec", [NQG, S], F32, False)
    pf_d = nc.declare_dram_parameter("prefv", [HD, NQG * NH], F32, False)
    ct_d = nc.declare_dram_parameter("cnt", [NH, NQG], F32, False)
    bm_d = nc.declare_dram_parameter("bandm", [P, P], F32, False)
    if with_bqk:
        bq_d = nc.declare_dram_parameter("bq", [P, 2], F32, False)
        bk_d = nc.declare_dram_parameter("bk", [P, 2], F32, False)
    if with_bv:
        bvb_d = nc.declare_dram_parameter("bvb", [1, HGD], F32, False)
    out_d = nc.declare_dram_parameter("outp", [S, DM], F32, True)

    KO = DM // P  # 4 k-sub-chunks for DM-contraction
    VW = HD + 1   # 65: V columns plus ones column

    with tile.TileContext(nc) as tc:
        with (
            tc.tile_pool(name="const", bufs=1) as const,
            tc.tile_pool(name="ppool", bufs=2, space="PSUM") as ppool,
            tc.tile_pool(name="spool", bufs=2, space="PSUM") as spool,
            tc.tile_pool(name="cpool", bufs=2, space="PSUM") as cpool,
            tc.tile_pool(name="ptp", bufs=4) as ptp,
            tc.tile_pool(name="ktsp", bufs=3) as ktsp,
            tc.tile_pool(name="bvqp", bufs=3) as bvqp,
            tc.tile_pool(name="ctxp", bufs=8) as ctxp,
            tc.tile_pool(name="densp", bufs=3) as densp,
        ):
            # ---- constant loads (weights first; x chunked by seq slice) ----
            wq_sb = const.tile([P, KO, HGD], MDT)
            nc.sync.dma_start(wq_sb, wq_d[:].rearrange("(ko p) m -> p ko m", p=P))
            wk_sb = const.tile([P, KO, HGD], MDT)
            nc.sync.dma_start(wk_sb, wk_d[:].rearrange("(ko p) m -> p ko m", p=P))
            wv_sb = const.tile([P, KO, HGD], MDT)
            nc.sync.dma_start(wv_sb, wv_d[:].rearrange("(ko p) m -> p ko m", p=P))
            xT_sb = const.tile([P, KO, S], MDT)
            xT_r = xT_d[:].rearrange("(ko p) s -> p ko s", p=P)
            for ns in range(4):
                nc.sync.dma_start(xT_sb[:, :, ns * QG:(ns + 1) * QG],
                                  xT_r[:, :, ns * QG:(ns + 1) * QG])
            # head on the free axis so every head's rhs sits at partition 0
            wo_sb = const.tile([HD, NH, DM], MDT)
            nc.sync.dma_start(wo_sb, wo_d[:].rearrange("(h p) n -> p h n", p=HD))

            avec_full = const.tile([P, S], F32)
            nc.sync.dma_start(avec_full, av_d[:].to_broadcast([P, S]))
            pft_sb = const.tile([HD, NQG * NH], F32)
            nc.sync.dma_start(pft_sb, pf_d[:])
            cnt_sb = const.tile([NH, NQG], F32)
            nc.sync.dma_start(cnt_sb, ct_d[:])
            bm_sb = const.tile([P, P], F32)
            nc.sync.dma_start(bm_sb, bm_d[:])
            eye4_sb = const.tile([NH, NH], F32)
            make_identity(nc, eye4_sb)
            eye128_sb = const.tile([P, P], F32)
            make_identity(nc, eye128_sb)
            if with_bqk:
                bq_sb = const.tile([P, 2], F32)
                nc.sync.dma_start(bq_sb, bq_d[:])
                bk_sb = const.tile([P, 2], F32)
                nc.sync.dma_start(bk_sb, bk_d[:])
            if with_bv:
                bv_full = const.tile([P, HGD], F32)
                nc.sync.dma_start(bv_full, bvb_d[:].to_broadcast([P, HGD]))

            # ---- projections (emitted per q-group, interleaved with the
            # ACT-bound attention stream so the PE stays dense) ----
            qT_f32 = const.tile([P, 2, S], F32)
            kT_sb = const.tile([P, 2, S], F32)
            qT_sb = const.tile([P, 2, S], MDT, name='qT_cast') if use_bf16 else qT_f32
            va_sb = const.tile([P, NKC, NH * VW], MDT)
            va_resh = va_sb.rearrange("p s (h c) -> p s h c", c=VW)
            nc.vector.memset(va_resh[:, :, :, HD], 1.0)

            def proj(ns):
                """q/k/v projections for sequence slice ns (one q-group)."""
                sl = slice(ns * QG, (ns + 1) * QG)
                for w_sb, t_sb, b_sb in (
                    (wq_sb, qT_f32, "q"),
                    (wk_sb, kT_sb, "k"),
                ):
                    for mc in range(2):
                        ps = ppool.tile([P, QG], F32, tag="pp")
                        for ki in range(KO):
                            nc.tensor.matmul(
                                ps,
                                lhsT=w_sb[:, ki, mc * P:(mc + 1) * P],
                                rhs=xT_sb[:, ki, sl],
                                start=(ki == 0),
                                stop=(ki == KO - 1),
                            )
                        if with_bqk:
                            bias = (bq_sb if b_sb == "q" else bk_sb)[:, mc:mc + 1]
                            nc.scalar.activation(
                                t_sb[:, mc, sl], ps,
                                mybir.ActivationFunctionType.Identity,
                                bias=bias,
                            )
                        else:
                            nc.scalar.copy(t_sb[:, mc, sl], ps)
                # fold a (and 1/sqrt(HD)) into q^T on the idle GPSIMD
                for mc in range(2):
                    nc.gpsimd.tensor_tensor(
                        qT_sb[:, mc, sl], qT_f32[:, mc, sl],
                        avec_full[:, sl], mybir.AluOpType.mult,
                    )
                for sc in range(4 * ns, 4 * ns + 4):
                    ps = ppool.tile([P, HGD], F32, tag="pp")
                    for ki in range(KO):
                        nc.tensor.matmul(
                            ps,
                            lhsT=xT_sb[:, ki, sc * P:(sc + 1) * P],
                            rhs=wv_sb[:, ki, :],
                            start=(ki == 0),
                            stop=(ki == KO - 1),
                        )
                    for h in range(NH):
                        dst = va_sb[:, sc, h * VW:h * VW + HD]
                        src = ps[:, h * HD:(h + 1) * HD]
                        if with_bv:
                            nc.vector.tensor_tensor(
                                dst, src, bv_full[:, h * HD:(h + 1) * HD],
                                mybir.AluOpType.add,
                            )
                        else:
                            nc.vector.tensor_copy(dst, src)

            # ---- attention + out-projection: software-pipelined q-groups --
            def prep(qg):
                """b-vector broadcast DMA + b-scaled k^T for group qg."""
                klo = kc_lo[qg] * KC
                khi = (qg + 1) * QG
                kw = khi - klo
                bvf = bvqp.tile([P, wmax], F32, tag="bvf")
                nc.sync.dma_start(
                    bvf[:, :kw],
                    bv_d[:][qg:qg + 1, klo:khi].to_broadcast([P, kw]),
                )
                kts = ktsp.tile([P, 2, wmax], MDT, tag="kts")
                for mc in range(2):
                    nc.gpsimd.tensor_tensor(
                        kts[:, mc, :kw], kT_sb[:, mc, klo:khi], bvf[:, :kw],
                        mybir.AluOpType.mult,
                    )
                return kts

            def attn(qg, kts):
                """score/exp/PV chains for all heads; returns ctx psums+dens."""
                klo = kc_lo[qg] * KC
                ctxps = [None] * NH
                dens = densp.tile([NH, QG], F32, tag="dens")
                kcs = list(range(kc_lo[qg], 4 * (qg + 1)))
                for hp in range(2):
                    # two heads of one 128-row kT chunk run CONCURRENTLY on
                    # the PE via row-tiling (array rows 0-63 / 64-127), and
                    # share one 2-bank score tile + one wide exp
                    h0, h1 = 2 * hp, 2 * hp + 1
                    cps_pair = []
                    for h in (h0, h1):
                        cps = cpool.tile([VW, QG], F32, tag="ctx")
                        cps_pair.append(cps)
                    for kc in kcs:
                        q_off = max(0, KC * (kc - 4 * qg))
                        co = kc * KC - klo
                        sp2 = spool.tile([P, 2, QG], F32, tag="spsum")
                        for j, h in enumerate((h0, h1)):
                            pb = (h % 2) * HD
                            nc.tensor.matmul(
                                sp2[:, j, q_off:],
                                lhsT=kts[pb:pb + HD, hp, co:co + KC],
                                rhs=qT_sb[pb:pb + HD, hp,
                                          qg * QG + q_off:(qg + 1) * QG],
                                start=True,
                                stop=True,
                            )
                        if kc >= 4 * qg:  # diagonal: mask both heads' bands
                            band = bass.AP(
                                tensor=sp2.tensor, offset=sp2.offset + q_off,
                                ap=[list(sp2.ap[0]), [QG, 2], [1, KC]],
                            )
                            nc.vector.tensor_tensor(
                                band, band, bm_sb[:, None, :].to_broadcast(
                                    [P, 2, KC]),
                                mybir.AluOpType.add,
                            )
                        pt = ptp.tile([P, 2, QG], MDT, tag="pt")
                        nc.scalar.activation(
                            pt[:, :, q_off:], sp2[:, :, q_off:],
                            mybir.ActivationFunctionType.Exp,
                        )
                        for j, h in enumerate((h0, h1)):
                            nc.tensor.matmul(
                                cps_pair[j][:, q_off:],
                                lhsT=va_sb[:, kc, h * VW:(h + 1) * VW],
                                rhs=pt[:, j, q_off:],
                                start=(kc == kcs[0]),
                                stop=(kc == kcs[-1]),
                            )
                    for j, h in enumerate((h0, h1)):
                        cps = cps_pair[j]
                        # denominator (PSUM partition 64) -> SBUF -> row h
                        d64 = densp.tile([HD + 1, QG], F32, tag="d64")
                        nc.vector.tensor_copy(d64[HD:HD + 1, :],
                                              cps[HD:HD + 1, :])
                        nc.sync.dma_start(dens[h:h + 1, :], d64[HD:HD + 1, :])
                        # undivided ctx to SBUF, freeing the accumulation bank
                        cxf = ctxp.tile([HD, QG], F32, tag="cxf")
                        nc.vector.tensor_copy(cxf, cps[:HD, :])
                        ctxps[h] = cxf
                # add the distant-past count to the denominators
                nc.vector.tensor_scalar_add(dens, dens, cnt_sb[:, qg:qg + 1])
                return ctxps, dens

            def tail(qg, ctxps, dens):
                """reciprocal (via PE transpose + 128-lane DVE), divide ctx,
                out-project, store."""
                dtp = ppool.tile([P, NH * (QG // P)], F32, tag="pp")
                for ss in range(QG // P):
                    nc.tensor.matmul(
                        dtp[:, ss * NH:(ss + 1) * NH],
                        lhsT=dens[:, ss * P:(ss + 1) * P],
                        rhs=eye4_sb,
                        start=True,
                        stop=True,
                    )
                rct = densp.tile([P, NH * (QG // P)], F32, tag="rct")
                nc.vector.reciprocal(rct, dtp)
                rps = ppool.tile([NH, QG], F32, tag="pp")
                for ss in range(QG // P):
                    nc.tensor.matmul(
                        rps[:, ss * P:(ss + 1) * P],
                        lhsT=rct[:, ss * NH:(ss + 1) * NH],
                        rhs=eye128_sb,
                        start=True,
                        stop=True,
                    )
                rec = densp.tile([NH, QG], F32, tag="rec")
                nc.vector.tensor_copy(rec, rps)
                ctx_sb = []
                for h in range(NH):
                    recl = densp.tile([1, QG], F32, tag="recl")
                    nc.sync.dma_start(recl, rec[h:h + 1, :])
                    bcs = densp.tile([HD, QG], F32, tag="bcs")
                    nc.gpsimd.partition_broadcast(bcs, recl)
                    csb = ctxp.tile([HD, QG], MDT, tag="ctxsb")
                    ctx_sb.append(csb)
                    # ctx = (near_ctx + distant_prefix) * (1/den)
                    nc.vector.scalar_tensor_tensor(
                        csb, ctxps[h],
                        pft_sb[:HD, qg * NH + h:qg * NH + h + 1],
                        bcs,
                        mybir.AluOpType.add,
                        mybir.AluOpType.mult,
                    )
                for ss in range(QG // P):
                    ops = ppool.tile([P, DM], F32, tag="pp")
                    for h in range(NH):
                        nc.tensor.matmul(
                            ops,
                            lhsT=ctx_sb[h][:, ss * P:(ss + 1) * P],
                            rhs=wo_sb[:, h, :],
                            start=(h == 0),
                            stop=(h == NH - 1),
                        )
                    osb = ptp.tile([P, DM], F32, tag="osb")
                    nc.vector.tensor_copy(osb, ops)
                    nc.sync.dma_start(
                        out_d[:][qg * QG + ss * P:qg * QG + (ss + 1) * P, :],
                        osb,
                    )

            # two-group lookahead: projections + scaled-k prefetch run well
            # ahead of the attention group that consumes them
            proj(0)
            ktss = [prep(0)]
            proj(1)
            ktss.append(prep(1))
            pending = None
            for qg in range(NQG):
                ctxps, dens = attn(qg, ktss[qg])
                if qg + 2 < NQG:
                    proj(qg + 2)
                    ktss.append(prep(qg + 2))
                if pending is not None:
                    tail(*pending)
                pending = (qg, ctxps, dens)
            tail(*pending)

    nc.finalize()
    return nc


# --------------------------------------------------------------------------
# host wrapper
# --------------------------------------------------------------------------

def _is_tril(mask: np.ndarray) -> bool:
    tril = np.tril(np.ones((S, S), dtype=mask.dtype))
    return all(np.array_equal(mask[b], tril) for b in range(mask.shape[0]))


def _prep_core_inputs(x, days, Wq, bq, Wk, bk, Wv, bv, Wo, rate,
                      use_bf16):
    """Per-core in_maps plus static loop bounds (shared across cores)."""
    t = days.astype(np.float64)  # [B, S]
    # distance beyond which exp(s * decay) == 1.0f exactly: need
    # |s| * exp(-rate*d) < 2^-25 with a generous |s| <= 150 bound.
    d_cut = (np.log(150.0) + 25.5 * np.log(2.0)) / rate
    # static near-window bounds (min over batches so one program fits all)
    kc_lo = []
    for qg in range(NQG):
        lo = NKC
        for b in range(B):
            tq = t[b, qg * QG]
            c = 0
            while c < 4 * qg and t[b, c * KC + KC - 1] < tq - d_cut:
                c += 1
            lo = min(lo, c)
        kc_lo.append(lo)
    kc_lo = tuple(kc_lo)
    wmax = max((qg + 1) * QG - kc_lo[qg] * KC for qg in range(NQG))
    wmax = ((wmax + P - 1) // P) * P

    # per-batch decay factor vectors (f64 for exactness, then f32)
    scale = 1.0 / np.sqrt(HD)
    t0 = np.stack([(t[:, qg * QG] + t[:, qg * QG + QG - 1]) * 0.5
                   for qg in range(NQG)], axis=1)  # [B, NQG]
    avec = np.zeros((B, 1, S), np.float32)
    bvec = np.zeros((B, NQG, S), np.float32)
    for b in range(B):
        for qg in range(NQG):
            sl = slice(qg * QG, (qg + 1) * QG)
            avec[b, 0, sl] = (np.exp(-rate * (t[b, sl] - t0[b, qg])) * scale
                              ).astype(np.float32)
            hi = (qg + 1) * QG
            bvec[b, qg, :hi] = (np.exp(rate * (t[b, :hi] - t0[b, qg]))
                                ).astype(np.float32)
    assert np.all(np.isfinite(avec)) and np.all(np.isfinite(bvec)), \
        "decay factor overflow; q-group span too large for fast path"

    # band mask: keep (0.0) iff q_local >= k_local else -1e30
    kl = np.arange(P)[:, None]
    ql = np.arange(P)[None, :]
    bandm = np.where(ql >= kl, 0.0, NEG).astype(np.float32)

    with_bqk = bool(np.any(bq != 0) or np.any(bk != 0))
    with_bv = bool(np.any(bv != 0))

    in_maps = []
    for c in range(NCORES):
        b, hg = divmod(c, NHG)
        cols = slice(hg * HGD, (hg + 1) * HGD)
        # prefix V sums for the distant rank-1 update: [HD, NQG*NH] (hd-major)
        prefv = np.zeros((HD, NQG * NH), np.float32)
        cnt = np.zeros((NH, NQG), np.float32)
        for qg in range(NQG):
            n = kc_lo[qg] * KC
            cnt[:, qg] = float(n)
            if n > 0:
                xs = x[b, :n].astype(np.float64).sum(axis=0)  # [DM]
                vs = xs @ Wv[cols, :].astype(np.float64).T \
                    + n * bv[cols].astype(np.float64)
                for h in range(NH):
                    prefv[:, qg * NH + h] = \
                        vs[h * HD:(h + 1) * HD].astype(np.float32)
        mdt = np.dtype(ml_dtypes.bfloat16) if use_bf16 else np.float32
        m = {
            "xT": np.ascontiguousarray(x[b].T).astype(mdt),
            "wqT": np.ascontiguousarray(Wq[cols, :].T).astype(mdt),
            "wkT": np.ascontiguousarray(Wk[cols, :].T).astype(mdt),
            "wvT": np.ascontiguousarray(Wv[cols, :].T).astype(mdt),
            "woT": np.ascontiguousarray(Wo[:, cols].T).astype(mdt),
            "avec": avec[b],
            "bvec": bvec[b],
            "prefv": prefv,
            "cnt": cnt,
            "bandm": bandm,
        }
        if with_bqk:
            m["bq"] = np.ascontiguousarray(
                bq[cols].reshape(2, P).T).astype(np.float32)
            m["bk"] = np.ascontiguousarray(
                bk[cols].reshape(2, P).T).astype(np.float32)
        if with_bv:
            m["bvb"] = bv[cols].reshape(1, HGD).astype(np.float32)
        in_maps.append(m)
    return in_maps, kc_lo, wmax, with_bqk, with_bv


def _reference_host(x, mask, days_offset, Wq, bq, Wk, bk, Wv, bv, Wo, bo,
                    decay_rate):
    """Emergency numpy fallback for inputs outside the fast path."""
    b, s, _ = x.shape
    out = np.empty((b, s, DM), np.float32)
    for bi in range(b):
        q = (x[bi] @ Wq.T + bq).reshape(s, H, HD).transpose(1, 0, 2)
        k = (x[bi] @ Wk.T + bk).reshape(s, H, HD).transpose(1, 0, 2)
        v = (x[bi] @ Wv.T + bv).reshape(s, H, HD).transpose(1, 0, 2)
        dist = np.abs(days_offset[bi][:, None] - days_offset[bi][None, :])
        decay = np.exp(-decay_rate * dist).astype(np.float32)
        ctx = np.empty((H, s, HD), np.float32)
        for h in range(H):
            sc = (q[h] @ k[h].T) / np.sqrt(HD) * decay
            sc = np.where(mask[bi] == 0, -np.inf, sc)
            sc = sc - sc.max(axis=-1, keepdims=True)
            e = np.exp(sc)
            ctx[h] = (e / e.sum(axis=-1, keepdims=True)) @ v[h]
        out[bi] = ctx.transpose(1, 0, 2).reshape(s, DM) @ Wo.T + bo
    return out


def kernel(x, mask, days_offset, Wq, bq, Wk, bk, Wv, bv, Wo, bo, decay_rate,
           _trace=False):
    x = np.asarray(x, np.float32)
    mask = np.asarray(mask)
    days = np.asarray(days_offset, np.float32)
    Wq, bq = np.asarray(Wq, np.float32), np.asarray(bq, np.float32)
    Wk, bk = np.asarray(Wk, np.float32), np.asarray(bk, np.float32)
    Wv, bv = np.asarray(Wv, np.float32), np.asarray(bv, np.float32)
    Wo, bo = np.asarray(Wo, np.float32), np.asarray(bo, np.float32)
    rate = float(np.asarray(decay_rate))

    sorted_ok = bool(np.all(np.diff(days, axis=-1) >= 0))
    if not (sorted_ok and _is_tril(mask)):
        return _reference_host(x, mask, days, Wq, bq, Wk, bk, Wv, bv, Wo, bo,
                               rate)

    use_bf16 = os.environ.get("KERNEL_F32", "") != "1"
    in_maps, kc_lo, wmax, with_bqk, with_bv = _prep_core_inputs(
        x, days, Wq, bq, Wk, bk, Wv, bv, Wo, rate, use_bf16)

    key = (kc_lo, wmax, with_bqk, with_bv, use_bf16)
    if key not in _cache:
        _cache[key] = _build_fast(kc_lo, wmax, with_bqk, with_bv, use_bf16)
    nc = _cache[key]

    res = run_bass_kernel_spmd(nc, in_maps, core_ids=list(range(NCORES)),
                               trace=_trace)
    out = np.empty((B, S, DM), np.float32)
    for b in range(B):
        out[b] = res.results[2 * b]["outp"] + res.results[2 * b + 1]["outp"] + bo
    if _trace:
        return out, res
    return out


# revision 39
# speedup vs baseline: 1.0229x; 1.0229x over previous
"""Trainium2 Bass kernel for temporal-decay causal multi-head attention.

Problem: nn_MultiHeadAttention_9053791060340
  B=4, S=2048, DM=512, H=8, HD=64.
  out = softmax((Q K^T / sqrt(HD)) * exp(-rate*|t_i - t_j|) with causal mask) V,
  then out-projection.

Sharding: 8 cores = 4 batches x 2 head-groups (4 heads each). Each core
computes a partial out-projection [S, DM] for its head group; the host sums
the two partials per batch and adds the output bias.

Device algorithm (per core); matmul inputs in bf16 with fp32 PSUM
accumulation by default (set KERNEL_F32=1 for all-fp32, ~2.2x slower,
error ~1e-6 instead of ~3e-3 scale-relative):
  - scores are computed TRANSPOSED (S^T[k, q] = k . q), so softmax-free-axis
    tricks are unnecessary: we use a no-max softmax (scores here are bounded,
    |v| <~ 64, so exp never overflows in fp32), and the denominator comes for
    free from a ones-column appended to V (PV matmul directly accumulates
    ctx^T[hd, q] plus the row of denominators).
  - temporal decay factorizes on sorted days: exp(-r(t_i - t_k)) = a_i * b_k
    with a per-q-group reference t0 to keep the factors in fp32 range.
    a (and the 1/sqrt(HD) scale) is folded into q^T once; b is folded into a
    per-q-group scaled copy of k^T.
  - pairs far enough apart (rate*dist > ~26) have s*decay so small that
    exp(s*decay) == 1.0f exactly (reference behaves identically), so all
    k-chunks entirely below the cutoff collapse into a rank-1 update
    ctx^T += PrefVsum x ones, with PrefVsum precomputed on the host.
  - causal masking needs work only in the diagonal 128x128 band of each
    q-group: an additive -1e30 tril tile is applied to S^T in PSUM before the
    exp. Blocks fully above the diagonal are skipped by restricting the
    streamed q-range.
"""

import os

import ml_dtypes
import numpy as np

import concourse.bass as bass
import concourse.tile as tile
from concourse import bacc
from concourse import mybir
from concourse.bass_utils import run_bass_kernel_spmd
from concourse.masks import make_identity

F32 = mybir.dt.float32

B, S, DM, H = 4, 2048, 512, 8
HD = DM // H          # 64
NCORES = 8
NHG = 2               # head groups == cores per batch
NH = H // NHG         # heads per core
HGD = NH * HD         # 256 output dims per core
QG = 512              # q-group width
NQG = S // QG         # 4
KC = 128              # k chunk (partition dim of S^T)
NKC = S // KC         # 16
P = 128
NEG = -1.0e30

_cache: dict = {}


# --------------------------------------------------------------------------
# device program
# --------------------------------------------------------------------------

def _build_fast(kc_lo: tuple, wmax: int, with_bqk: bool, with_bv: bool,
                use_bf16: bool = True):
    """Build the SPMD Bass program.

    kc_lo[qg] = first near k-chunk per q-group (static across cores; chunks
    below it are covered by the prefix-sum rank-1 update). wmax = max
    near-window width in elements for the scaled-k tile. use_bf16 casts all
    matmul inputs to bf16 (fp32 PSUM accumulation): the PE streams 1 col/cyc
    for bf16 vs 2 for fp32, halving matmul time.

    The q-group loop is software-pipelined: prep(qg+1) (b-vector DMA +
    scaled-k) is emitted before tail(qg) (reciprocal, divide, out-project)
    so the PE always has next-group score matmuls available while the
    denominator pipeline of the previous group drains.
    """
    nc = bacc.Bacc()
    MDT = mybir.dt.bfloat16 if use_bf16 else F32

    xT_d = nc.declare_dram_parameter("xT", [DM, S], MDT, False)
    wq_d = nc.declare_dram_parameter("wqT", [DM, HGD], MDT, False)
    wk_d = nc.declare_dram_parameter("wkT", [DM, HGD], MDT, False)
    wv_d = nc.declare_dram_parameter("wvT", [DM, HGD], MDT, False)
    wo_d = nc.declare_dram_parameter("woT", [HGD, DM], MDT, False)
    av_d = nc.declare_dram_parameter("avec", [1, S], F32, False)
    bv_d = nc.declare_dram_parameter("bvec", [NQG, S], F32, False)
    pf_d = nc.declare_dram_parameter("prefv", [HD, NQG * NH], F32, False)
    ct_d = nc.declare_dram_parameter("cnt", [NH, NQG], F32, False)
    bm_d = nc.declare_dram_parameter("bandm", [P, P], F32, False)
    if with_bqk:
        bq_d = nc.declare_dram_parameter("bq", [P, 2], F32, False)
        bk_d = nc.declare_dram_parameter("bk", [P, 2], F32, False)
    if with_bv:
        bvb_d = nc.declare_dram_parameter("bvb", [1, HGD], F32, False)
    out_d = nc.declare_dram_parameter("outp", [S, DM], F32, True)

    KO = DM // P  # 4 k-sub-chunks for DM-contraction
    VW = HD + 1   # 65: V columns plus ones column

    with tile.TileContext(nc) as tc:
        with (
            tc.tile_pool(name="const", bufs=1) as const,
            tc.tile_pool(name="ppool", bufs=2, space="PSUM") as ppool,
            tc.tile_pool(name="spool", bufs=2, space="PSUM") as spool,
            tc.tile_pool(name="cpool", bufs=2, space="PSUM") as cpool,
            tc.tile_pool(name="ptp", bufs=4) as ptp,
            tc.tile_pool(name="ktsp", bufs=3) as ktsp,
            tc.tile_pool(name="bvqp", bufs=3) as bvqp,
            tc.tile_pool(name="ctxp", bufs=8) as ctxp,
            tc.tile_pool(name="densp", bufs=3) as densp,
        ):
            # ---- constant loads (weights first; x chunked by seq slice) ----
            wq_sb = const.tile([P, KO, HGD], MDT)
            nc.sync.dma_start(wq_sb, wq_d[:].rearrange("(ko p) m -> p ko m", p=P))
            wk_sb = const.tile([P, KO, HGD], MDT)
            nc.sync.dma_start(wk_sb, wk_d[:].rearrange("(ko p) m -> p ko m", p=P))
            wv_sb = const.tile([P, KO, HGD], MDT)
            nc.sync.dma_start(wv_sb, wv_d[:].rearrange("(ko p) m -> p ko m", p=P))
            xT_sb = const.tile([P, KO, S], MDT)
            xT_r = xT_d[:].rearrange("(ko p) s -> p ko s", p=P)
            for ns in range(4):
                nc.sync.dma_start(xT_sb[:, :, ns * QG:(ns + 1) * QG],
                                  xT_r[:, :, ns * QG:(ns + 1) * QG])
            # head on the free axis so every head's rhs sits at partition 0
            wo_sb = const.tile([HD, NH, DM], MDT)
            nc.sync.dma_start(wo_sb, wo_d[:].rearrange("(h p) n -> p h n", p=HD))

            avec_full = const.tile([P, S], F32)
            nc.sync.dma_start(avec_full, av_d[:].to_broadcast([P, S]))
            pft_sb = const.tile([HD, NQG * NH], F32)
            nc.sync.dma_start(pft_sb, pf_d[:])
            cnt_sb = const.tile([NH, NQG], F32)
            nc.sync.dma_start(cnt_sb, ct_d[:])
            bm_sb = const.tile([P, P], F32)
            nc.sync.dma_start(bm_sb, bm_d[:])
            eye4_sb = const.tile([NH, NH], F32)
            make_identity(nc, eye4_sb)
            eye128_sb = const.tile([P, P], F32)
            make_identity(nc, eye128_sb)
            if with_bqk:
                bq_sb = const.tile([P, 2], F32)
                nc.sync.dma_start(bq_sb, bq_d[:])
                bk_sb = const.tile([P, 2], F32)
                nc.sync.dma_start(bk_sb, bk_d[:])
            if with_bv:
                bv_full = const.tile([P, HGD], F32)
                nc.sync.dma_start(bv_full, bvb_d[:].to_broadcast([P, HGD]))

            # ---- projections (emitted per q-group, interleaved with the
            # ACT-bound attention stream so the PE stays dense) ----
            qT_f32 = const.tile([P, 2, S], F32)
            kT_sb = const.tile([P, 2, S], F32)
            qT_sb = const.tile([P, 2, S], MDT, name='qT_cast') if use_bf16 else qT_f32
            va_sb = const.tile([P, NKC, NH * VW], MDT)
            va_resh = va_sb.rearrange("p s (h c) -> p s h c", c=VW)
            nc.vector.memset(va_resh[:, :, :, HD], 1.0)

            def proj(ns):
                """q/k/v projections for sequence slice ns (one q-group)."""
                sl = slice(ns * QG, (ns + 1) * QG)
                for w_sb, t_sb, b_sb in (
                    (wq_sb, qT_f32, "q"),
                    (wk_sb, kT_sb, "k"),
                ):
                    for mc in range(2):
                        ps = ppool.tile([P, QG], F32, tag="pp")
                        for ki in range(KO):
                            nc.tensor.matmul(
                                ps,
                                lhsT=w_sb[:, ki, mc * P:(mc + 1) * P],
                                rhs=xT_sb[:, ki, sl],
                                start=(ki == 0),
                                stop=(ki == KO - 1),
                            )
                        if with_bqk:
                            bias = (bq_sb if b_sb == "q" else bk_sb)[:, mc:mc + 1]
                            nc.scalar.activation(
                                t_sb[:, mc, sl], ps,
                                mybir.ActivationFunctionType.Identity,
                                bias=bias,
                            )
                        else:
                            nc.scalar.copy(t_sb[:, mc, sl], ps)
                # fold a (and 1/sqrt(HD)) into q^T on the idle GPSIMD
                for mc in range(2):
                    nc.gpsimd.tensor_tensor(
                        qT_sb[:, mc, sl], qT_f32[:, mc, sl],
                        avec_full[:, sl], mybir.AluOpType.mult,
                    )
                for sc in range(4 * ns, 4 * ns + 4):
                    ps = ppool.tile([P, HGD], F32, tag="pp")
                    for ki in range(KO):
                        nc.tensor.matmul(
                            ps,
                            lhsT=xT_sb[:, ki, sc * P:(sc + 1) * P],
                            rhs=wv_sb[:, ki, :],
                            start=(ki == 0),
                            stop=(ki == KO - 1),
                        )
                    for h in range(NH):
                        dst = va_sb[:, sc, h * VW:h * VW + HD]
                        src = ps[:, h * HD:(h + 1) * HD]
                        if with_bv:
                            nc.vector.tensor_tensor(
                                dst, src, bv_full[:, h * HD:(h + 1) * HD],
                                mybir.AluOpType.add,
                            )
                        else:
                            nc.vector.tensor_copy(dst, src)

            # ---- attention + out-projection: software-pipelined q-groups --
            def prep(qg):
                """b-vector broadcast DMA + b-scaled k^T for group qg."""
                klo = kc_lo[qg] * KC
                khi = (qg + 1) * QG
                kw = khi - klo
                bvf = bvqp.tile([P, wmax], F32, tag="bvf")
                nc.sync.dma_start(
                    bvf[:, :kw],
                    bv_d[:][qg:qg + 1, klo:khi].to_broadcast([P, kw]),
                )
                kts = ktsp.tile([P, 2, wmax], MDT, tag="kts")
                for mc in range(2):
                    nc.gpsimd.tensor_tensor(
                        kts[:, mc, :kw], kT_sb[:, mc, klo:khi], bvf[:, :kw],
                        mybir.AluOpType.mult,
                    )
                return kts

            def attn(qg, kts):
                """score/exp/PV chains for all heads; returns ctx psums+dens."""
                klo = kc_lo[qg] * KC
                ctxps = [None] * NH
                dens = densp.tile([NH, QG], F32, tag="dens")
                kcs = list(range(kc_lo[qg], 4 * (qg + 1)))
                for hp in range(2):
                    # two heads of one 128-row kT chunk run CONCURRENTLY on
                    # the PE via row-tiling (array rows 0-63 / 64-127), and
                    # share one 2-bank score tile + one wide exp
                    h0, h1 = 2 * hp, 2 * hp + 1
                    cps_pair = []
                    for h in (h0, h1):
                        cps = cpool.tile([VW, QG], F32, tag="ctx")
                        cps_pair.append(cps)
                    for kc in kcs:
                        q_off = max(0, KC * (kc - 4 * qg))
                        co = kc * KC - klo
                        sp2 = spool.tile([P, 2, QG], F32, tag="spsum")
                        for j, h in enumerate((h0, h1)):
                            pb = (h % 2) * HD
                            nc.tensor.matmul(
                                sp2[:, j, q_off:],
                                lhsT=kts[pb:pb + HD, hp, co:co + KC],
                                rhs=qT_sb[pb:pb + HD, hp,
                                          qg * QG + q_off:(qg + 1) * QG],
                                start=True,
                                stop=True,
                            )
                        if kc >= 4 * qg:  # diagonal: mask both heads' bands
                            band = bass.AP(
                                tensor=sp2.tensor, offset=sp2.offset + q_off,
                                ap=[list(sp2.ap[0]), [QG, 2], [1, KC]],
                            )
                            nc.vector.tensor_tensor(
                                band, band, bm_sb[:, None, :].to_broadcast(
                                    [P, 2, KC]),
                                mybir.AluOpType.add,
                            )
                        pt = ptp.tile([P, 2, QG], MDT, tag="pt")
                        nc.scalar.activation(
                            pt[:, :, q_off:], sp2[:, :, q_off:],
                            mybir.ActivationFunctionType.Exp,
                        )
                        for j, h in enumerate((h0, h1)):
                            nc.tensor.matmul(
                                cps_pair[j][:, q_off:],
                                lhsT=va_sb[:, kc, h * VW:(h + 1) * VW],
                                rhs=pt[:, j, q_off:],
                                start=(kc == kcs[0]),
                                stop=(kc == kcs[-1]),
                            )
                    for j, h in enumerate((h0, h1)):
                        cps = cps_pair[j]
                        # denominator (PSUM partition 64) -> SBUF -> row h
                        d64 = densp.tile([HD + 1, QG], F32, tag="d64")
                        nc.vector.tensor_copy(d64[HD:HD + 1, :],
                                              cps[HD:HD + 1, :])
                        nc.sync.dma_start(dens[h:h + 1, :], d64[HD:HD + 1, :])
                        # undivided ctx to SBUF, freeing the accumulation bank
                        cxf = ctxp.tile([HD, QG], F32, tag="cxf")
                        nc.scalar.copy(cxf, cps[:HD, :])
                        ctxps[h] = cxf
                # add the distant-past count to the denominators
                nc.vector.tensor_scalar_add(dens, dens, cnt_sb[:, qg:qg + 1])
                return ctxps, dens

            def tail(qg, ctxps, dens):
                """reciprocal (via PE transpose + 128-lane DVE), divide ctx,
                out-project, store."""
                dtp = ppool.tile([P, NH * (QG // P)], F32, tag="pp")
                for ss in range(QG // P):
                    nc.tensor.matmul(
                        dtp[:, ss * NH:(ss + 1) * NH],
                        lhsT=dens[:, ss * P:(ss + 1) * P],
                        rhs=eye4_sb,
                        start=True,
                        stop=True,
                    )
                rct = densp.tile([P, NH * (QG // P)], F32, tag="rct")
                nc.vector.reciprocal(rct, dtp)
                rps = ppool.tile([NH, QG], F32, tag="pp")
                for ss in range(QG // P):
                    nc.tensor.matmul(
                        rps[:, ss * P:(ss + 1) * P],
                        lhsT=rct[:, ss * NH:(ss + 1) * NH],
                        rhs=eye128_sb,
                        start=True,
                        stop=True,
                    )
                rec = densp.tile([NH, QG], F32, tag="rec")
                nc.vector.tensor_copy(rec, rps)
                ctx_sb = []
                for h in range(NH):
                    recl = densp.tile([1, QG], F32, tag="recl")
                    nc.sync.dma_start(recl, rec[h:h + 1, :])
                    bcs = densp.tile([HD, QG], F32, tag="bcs")
                    nc.gpsimd.partition_broadcast(bcs, recl)
                    csb = ctxp.tile([HD, QG], MDT, tag="ctxsb")
                    ctx_sb.append(csb)
                    # ctx = (near_ctx + distant_prefix) * (1/den)
                    nc.vector.scalar_tensor_tensor(
                        csb, ctxps[h],
                        pft_sb[:HD, qg * NH + h:qg * NH + h + 1],
                        bcs,
                        mybir.AluOpType.add,
                        mybir.AluOpType.mult,
                    )
                for ss in range(QG // P):
                    ops = ppool.tile([P, DM], F32, tag="pp")
                    for h in range(NH):
                        nc.tensor.matmul(
                            ops,
                            lhsT=ctx_sb[h][:, ss * P:(ss + 1) * P],
                            rhs=wo_sb[:, h, :],
                            start=(h == 0),
                            stop=(h == NH - 1),
                        )
                    osb = ptp.tile([P, DM], F32, tag="osb")
                    nc.vector.tensor_copy(osb, ops)
                    nc.sync.dma_start(
                        out_d[:][qg * QG + ss * P:qg * QG + (ss + 1) * P, :],
                        osb,
                    )

            # two-group lookahead: projections + scaled-k prefetch run well
            # ahead of the attention group that consumes them
            proj(0)
            ktss = [prep(0)]
            proj(1)
            ktss.append(prep(1))
            pending = None
            for qg in range(NQG):
                ctxps, dens = attn(qg, ktss[qg])
                if qg + 2 < NQG:
                    proj(qg + 2)
                    ktss.append(prep(qg + 2))
                if pending is not None:
                    tail(*pending)
                pending = (qg, ctxps, dens)
            tail(*pending)

    nc.finalize()
    return nc


# --------------------------------------------------------------------------
# host wrapper
# --------------------------------------------------------------------------

def _is_tril(mask: np.ndarray) -> bool:
    tril = np.tril(np.ones((S, S), dtype=mask.dtype))
    return all(np.array_equal(mask[b], tril) for b in range(mask.shape[0]))


def _prep_core_inputs(x, days, Wq, bq, Wk, bk, Wv, bv, Wo, rate,
                      use_bf16):
    """Per-core in_maps plus static loop bounds (shared across cores)."""
    t = days.astype(np.float64)  # [B, S]
    # distance beyond which exp(s * decay) == 1.0f exactly: need
    # |s| * exp(-rate*d) < 2^-25 with a generous |s| <= 150 bound.
    d_cut = (np.log(150.0) + 25.5 * np.log(2.0)) / rate
    # static near-window bounds (min over batches so one program fits all)
    kc_lo = []
    for qg in range(NQG):
        lo = NKC
        for b in range(B):
            tq = t[b, qg * QG]
            c = 0
            while c < 4 * qg and t[b, c * KC + KC - 1] < tq - d_cut:
                c += 1
            lo = min(lo, c)
        kc_lo.append(lo)
    kc_lo = tuple(kc_lo)
    wmax = max((qg + 1) * QG - kc_lo[qg] * KC for qg in range(NQG))
    wmax = ((wmax + P - 1) // P) * P

    # per-batch decay factor vectors (f64 for exactness, then f32)
    scale = 1.0 / np.sqrt(HD)
    t0 = np.stack([(t[:, qg * QG] + t[:, qg * QG + QG - 1]) * 0.5
                   for qg in range(NQG)], axis=1)  # [B, NQG]
    avec = np.zeros((B, 1, S), np.float32)
    bvec = np.zeros((B, NQG, S), np.float32)
    for b in range(B):
        for qg in range(NQG):
            sl = slice(qg * QG, (qg + 1) * QG)
            avec[b, 0, sl] = (np.exp(-rate * (t[b, sl] - t0[b, qg])) * scale
                              ).astype(np.float32)
            hi = (qg + 1) * QG
            bvec[b, qg, :hi] = (np.exp(rate * (t[b, :hi] - t0[b, qg]))
                                ).astype(np.float32)
    assert np.all(np.isfinite(avec)) and np.all(np.isfinite(bvec)), \
        "decay factor overflow; q-group span too large for fast path"

    # band mask: keep (0.0) iff q_local >= k_local else -1e30
    kl = np.arange(P)[:, None]
    ql = np.arange(P)[None, :]
    bandm = np.where(ql >= kl, 0.0, NEG).astype(np.float32)

    with_bqk = bool(np.any(bq != 0) or np.any(bk != 0))
    with_bv = bool(np.any(bv != 0))

    in_maps = []
    for c in range(NCORES):
        b, hg = divmod(c, NHG)
        cols = slice(hg * HGD, (hg + 1) * HGD)
        # prefix V sums for the distant rank-1 update: [HD, NQG*NH] (hd-major)
        prefv = np.zeros((HD, NQG * NH), np.float32)
        cnt = np.zeros((NH, NQG), np.float32)
        for qg in range(NQG):
            n = kc_lo[qg] * KC
            cnt[:, qg] = float(n)
            if n > 0:
                xs = x[b, :n].astype(np.float64).sum(axis=0)  # [DM]
                vs = xs @ Wv[cols, :].astype(np.float64).T \
                    + n * bv[cols].astype(np.float64)
                for h in range(NH):
                    prefv[:, qg * NH + h] = \
                        vs[h * HD:(h + 1) * HD].astype(np.float32)
        mdt = np.dtype(ml_dtypes.bfloat16) if use_bf16 else np.float32
        m = {
            "xT": np.ascontiguousarray(x[b].T).astype(mdt),
            "wqT": np.ascontiguousarray(Wq[cols, :].T).astype(mdt),
            "wkT": np.ascontiguousarray(Wk[cols, :].T).astype(mdt),
            "wvT": np.ascontiguousarray(Wv[cols, :].T).astype(mdt),
            "woT": np.ascontiguousarray(Wo[:, cols].T).astype(mdt),
            "avec": avec[b],
            "bvec": bvec[b],
            "prefv": prefv,
            "cnt": cnt,
            "bandm": bandm,
        }
        if with_bqk:
            m["bq"] = np.ascontiguousarray(
                bq[cols].reshape(2, P).T).astype(np.float32)
            m["bk"] = np.ascontiguousarray(
                bk[cols].reshape(2, P).T).astype(np.float32)
        if with_bv:
            m["bvb"] = bv[cols].reshape(1, HGD).astype(np.float32)
        in_maps.append(m)
    return in_maps, kc_lo, wmax, with_bqk, with_bv


def _reference_host(x, mask, days_offset, Wq, bq, Wk, bk, Wv, bv, Wo, bo,
                    decay_rate):
    """Emergency numpy fallback for inputs outside the fast path."""
    b, s, _ = x.shape
    out = np.empty((b, s, DM), np.float32)
    for bi in range(b):
        q = (x[bi] @ Wq.T + bq).reshape(s, H, HD).transpose(1, 0, 2)
        k = (x[bi] @ Wk.T + bk).reshape(s, H, HD).transpose(1, 0, 2)
        v = (x[bi] @ Wv.T + bv).reshape(s, H, HD).transpose(1, 0, 2)
        dist = np.abs(days_offset[bi][:, None] - days_offset[bi][None, :])
        decay = np.exp(-decay_rate * dist).astype(np.float32)
        ctx = np.empty((H, s, HD), np.float32)
        for h in range(H):
            sc = (q[h] @ k[h].T) / np.sqrt(HD) * decay
            sc = np.where(mask[bi] == 0, -np.inf, sc)
            sc = sc - sc.max(axis=-1, keepdims=True)
            e = np.exp(sc)
            ctx[h] = (e / e.sum(axis=-1, keepdims=True)) @ v[h]
        out[bi] = ctx.transpose(1, 0, 2).reshape(s, DM) @ Wo.T + bo
    return out


def kernel(x, mask, days_offset, Wq, bq, Wk, bk, Wv, bv, Wo, bo, decay_rate,
           _trace=False):
    x = np.asarray(x, np.float32)
    mask = np.asarray(mask)
    days = np.asarray(days_offset, np.float32)
    Wq, bq = np.asarray(Wq, np.float32), np.asarray(bq, np.float32)
    Wk, bk = np.asarray(Wk, np.float32), np.asarray(bk, np.float32)
    Wv, bv = np.asarray(Wv, np.float32), np.asarray(bv, np.float32)
    Wo, bo = np.asarray(Wo, np.float32), np.asarray(bo, np.float32)
    rate = float(np.asarray(decay_rate))

    sorted_ok = bool(np.all(np.diff(days, axis=-1) >= 0))
    if not (sorted_ok and _is_tril(mask)):
        return _reference_host(x, mask, days, Wq, bq, Wk, bk, Wv, bv, Wo, bo,
                               rate)

    use_bf16 = os.environ.get("KERNEL_F32", "") != "1"
    in_maps, kc_lo, wmax, with_bqk, with_bv = _prep_core_inputs(
        x, days, Wq, bq, Wk, bk, Wv, bv, Wo, rate, use_bf16)

    key = (kc_lo, wmax, with_bqk, with_bv, use_bf16)
    if key not in _cache:
        _cache[key] = _build_fast(kc_lo, wmax, with_bqk, with_bv, use_bf16)
    nc = _cache[key]

    res = run_bass_kernel_spmd(nc, in_maps, core_ids=list(range(NCORES)),
                               trace=_trace)
    out = np.empty((B, S, DM), np.float32)
    for b in range(B):
        out[b] = res.results[2 * b]["outp"] + res.results[2 * b + 1]["outp"] + bo
    if _trace:
        return out, res
    return out


# revision 40
# speedup vs baseline: 1.0463x; 1.0229x over previous
"""Trainium2 Bass kernel for temporal-decay causal multi-head attention.

Problem: nn_MultiHeadAttention_9053791060340
  B=4, S=2048, DM=512, H=8, HD=64.
  out = softmax((Q K^T / sqrt(HD)) * exp(-rate*|t_i - t_j|) with causal mask) V,
  then out-projection.

Sharding: 8 cores = 4 batches x 2 head-groups (4 heads each). Each core
computes a partial out-projection [S, DM] for its head group; the host sums
the two partials per batch and adds the output bias.

Device algorithm (per core); matmul inputs in bf16 with fp32 PSUM
accumulation by default (set KERNEL_F32=1 for all-fp32, ~2.2x slower,
error ~1e-6 instead of ~3e-3 scale-relative):
  - scores are computed TRANSPOSED (S^T[k, q] = k . q), so softmax-free-axis
    tricks are unnecessary: we use a no-max softmax (scores here are bounded,
    |v| <~ 64, so exp never overflows in fp32), and the denominator comes for
    free from a ones-column appended to V (PV matmul directly accumulates
    ctx^T[hd, q] plus the row of denominators).
  - temporal decay factorizes on sorted days: exp(-r(t_i - t_k)) = a_i * b_k
    with a per-q-group reference t0 to keep the factors in fp32 range.
    a (and the 1/sqrt(HD) scale) is folded into q^T once; b is folded into a
    per-q-group scaled copy of k^T.
  - pairs far enough apart (rate*dist > ~26) have s*decay so small that
    exp(s*decay) == 1.0f exactly (reference behaves identically), so all
    k-chunks entirely below the cutoff collapse into a rank-1 update
    ctx^T += PrefVsum x ones, with PrefVsum precomputed on the host.
  - causal masking needs work only in the diagonal 128x128 band of each
    q-group: an additive -1e30 tril tile is applied to S^T in PSUM before the
    exp. Blocks fully above the diagonal are skipped by restricting the
    streamed q-range.
"""

import os

import ml_dtypes
import numpy as np

import concourse.bass as bass
import concourse.tile as tile
from concourse import bacc
from concourse import mybir
from concourse.bass_utils import run_bass_kernel_spmd
from concourse.masks import make_identity

F32 = mybir.dt.float32

B, S, DM, H = 4, 2048, 512, 8
HD = DM // H          # 64
NCORES = 8
NHG = 2               # head groups == cores per batch
NH = H // NHG         # heads per core
HGD = NH * HD         # 256 output dims per core
QG = 512              # q-group width
NQG = S // QG         # 4
KC = 128              # k chunk (partition dim of S^T)
NKC = S // KC         # 16
P = 128
NEG = -1.0e30

_cache: dict = {}


# --------------------------------------------------------------------------
# device program
# --------------------------------------------------------------------------

def _build_fast(kc_lo: tuple, wmax: int, with_bqk: bool, with_bv: bool,
                use_bf16: bool = True):
    """Build the SPMD Bass program.

    kc_lo[qg] = first near k-chunk per q-group (static across cores; chunks
    below it are covered by the prefix-sum rank-1 update). wmax = max
    near-window width in elements for the scaled-k tile. use_bf16 casts all
    matmul inputs to bf16 (fp32 PSUM accumulation): the PE streams 1 col/cyc
    for bf16 vs 2 for fp32, halving matmul time.

    The q-group loop is software-pipelined: prep(qg+1) (b-vector DMA +
    scaled-k) is emitted before tail(qg) (reciprocal, divide, out-project)
    so the PE always has next-group score matmuls available while the
    denominator pipeline of the previous group drains.
    """
    nc = bacc.Bacc()
    MDT = mybir.dt.bfloat16 if use_bf16 else F32

    xT_d = nc.declare_dram_parameter("xT", [DM, S], MDT, False)
    wq_d = nc.declare_dram_parameter("wqT", [DM, HGD], MDT, False)
    wk_d = nc.declare_dram_parameter("wkT", [DM, HGD], MDT, False)
    wv_d = nc.declare_dram_parameter("wvT", [DM, HGD], MDT, False)
    wo_d = nc.declare_dram_parameter("woT", [HGD, DM], MDT, False)
    av_d = nc.declare_dram_parameter("avec", [1, S], F32, False)
    bv_d = nc.declare_dram_parameter("bvec", [NQG, S], F32, False)
    pf_d = nc.declare_dram_parameter("prefv", [HD, NQG * NH], F32, False)
    ct_d = nc.declare_dram_parameter("cnt", [NH, NQG], F32, False)
    bm_d = nc.declare_dram_parameter("bandm", [P, P], F32, False)
    if with_bqk:
        bq_d = nc.declare_dram_parameter("bq", [P, 2], F32, False)
        bk_d = nc.declare_dram_parameter("bk", [P, 2], F32, False)
    if with_bv:
        bvb_d = nc.declare_dram_parameter("bvb", [1, HGD], F32, False)
    out_d = nc.declare_dram_parameter("outp", [S, DM], F32, True)

    KO = DM // P  # 4 k-sub-chunks for DM-contraction
    VW = HD + 1   # 65: V columns plus ones column

    with tile.TileContext(nc) as tc:
        with (
            tc.tile_pool(name="const", bufs=1) as const,
            tc.tile_pool(name="ppool", bufs=2, space="PSUM") as ppool,
            tc.tile_pool(name="spool", bufs=2, space="PSUM") as spool,
            tc.tile_pool(name="cpool", bufs=2, space="PSUM") as cpool,
            tc.tile_pool(name="ptp", bufs=3) as ptp,
            tc.tile_pool(name="ktsp", bufs=3) as ktsp,
            tc.tile_pool(name="bvqp", bufs=3) as bvqp,
            tc.tile_pool(name="ctxp", bufs=6) as ctxp,
            tc.tile_pool(name="densp", bufs=2) as densp,
        ):
            # ---- constant loads (weights first; x chunked by seq slice) ----
            wq_sb = const.tile([P, KO, HGD], MDT)
            nc.sync.dma_start(wq_sb, wq_d[:].rearrange("(ko p) m -> p ko m", p=P))
            wk_sb = const.tile([P, KO, HGD], MDT)
            nc.sync.dma_start(wk_sb, wk_d[:].rearrange("(ko p) m -> p ko m", p=P))
            wv_sb = const.tile([P, KO, HGD], MDT)
            nc.sync.dma_start(wv_sb, wv_d[:].rearrange("(ko p) m -> p ko m", p=P))
            xT_sb = const.tile([P, KO, S], MDT)
            xT_r = xT_d[:].rearrange("(ko p) s -> p ko s", p=P)
            for ns in range(4):
                nc.sync.dma_start(xT_sb[:, :, ns * QG:(ns + 1) * QG],
                                  xT_r[:, :, ns * QG:(ns + 1) * QG])
            # head on the free axis so every head's rhs sits at partition 0
            wo_sb = const.tile([HD, NH, DM], MDT)
            nc.sync.dma_start(wo_sb, wo_d[:].rearrange("(h p) n -> p h n", p=HD))

            avec_full = const.tile([P, S], F32)
            nc.sync.dma_start(avec_full, av_d[:].to_broadcast([P, S]))
            pft_sb = const.tile([HD, NQG * NH], F32)
            nc.sync.dma_start(pft_sb, pf_d[:])
            cnt_sb = const.tile([NH, NQG], F32)
            nc.sync.dma_start(cnt_sb, ct_d[:])
            bm_sb = const.tile([P, P], F32)
            nc.sync.dma_start(bm_sb, bm_d[:])
            eye4_sb = const.tile([NH, NH], F32)
            make_identity(nc, eye4_sb)
            eye128_sb = const.tile([P, P], F32)
            make_identity(nc, eye128_sb)
            if with_bqk:
                bq_sb = const.tile([P, 2], F32)
                nc.sync.dma_start(bq_sb, bq_d[:])
                bk_sb = const.tile([P, 2], F32)
                nc.sync.dma_start(bk_sb, bk_d[:])
            if with_bv:
                bv_full = const.tile([P, HGD], F32)
                nc.sync.dma_start(bv_full, bvb_d[:].to_broadcast([P, HGD]))

            # ---- projections (emitted per q-group, interleaved with the
            # ACT-bound attention stream so the PE stays dense) ----
            qT_f32 = const.tile([P, 2, S], F32)
            kT_sb = const.tile([P, 2, S], F32)
            qT_sb = const.tile([P, 2, S], MDT, name='qT_cast') if use_bf16 else qT_f32
            va_sb = const.tile([P, NKC, NH * VW], MDT)
            va_resh = va_sb.rearrange("p s (h c) -> p s h c", c=VW)
            nc.vector.memset(va_resh[:, :, :, HD], 1.0)

            def proj(ns):
                """q/k/v projections for sequence slice ns (one q-group)."""
                sl = slice(ns * QG, (ns + 1) * QG)
                for w_sb, t_sb, b_sb in (
                    (wq_sb, qT_f32, "q"),
                    (wk_sb, kT_sb, "k"),
                ):
                    for mc in range(2):
                        ps = ppool.tile([P, QG], F32, tag="pp")
                        for ki in range(KO):
                            nc.tensor.matmul(
                                ps,
                                lhsT=w_sb[:, ki, mc * P:(mc + 1) * P],
                                rhs=xT_sb[:, ki, sl],
                                start=(ki == 0),
                                stop=(ki == KO - 1),
                            )
                        if with_bqk:
                            bias = (bq_sb if b_sb == "q" else bk_sb)[:, mc:mc + 1]
                            nc.scalar.activation(
                                t_sb[:, mc, sl], ps,
                                mybir.ActivationFunctionType.Identity,
                                bias=bias,
                            )
                        else:
                            nc.scalar.copy(t_sb[:, mc, sl], ps)
                # fold a (and 1/sqrt(HD)) into q^T on the idle GPSIMD
                for mc in range(2):
                    nc.gpsimd.tensor_tensor(
                        qT_sb[:, mc, sl], qT_f32[:, mc, sl],
                        avec_full[:, sl], mybir.AluOpType.mult,
                    )
                for sc in range(4 * ns, 4 * ns + 4):
                    ps = ppool.tile([P, HGD], F32, tag="pp")
                    for ki in range(KO):
                        nc.tensor.matmul(
                            ps,
                            lhsT=xT_sb[:, ki, sc * P:(sc + 1) * P],
                            rhs=wv_sb[:, ki, :],
                            start=(ki == 0),
                            stop=(ki == KO - 1),
                        )
                    for h in range(NH):
                        dst = va_sb[:, sc, h * VW:h * VW + HD]
                        src = ps[:, h * HD:(h + 1) * HD]
                        if with_bv:
                            nc.vector.tensor_tensor(
                                dst, src, bv_full[:, h * HD:(h + 1) * HD],
                                mybir.AluOpType.add,
                            )
                        else:
                            nc.vector.tensor_copy(dst, src)

            # ---- attention + out-projection: software-pipelined q-groups --
            def prep(qg):
                """b-vector broadcast DMA + b-scaled k^T for group qg."""
                klo = kc_lo[qg] * KC
                khi = (qg + 1) * QG
                kw = khi - klo
                bvf = bvqp.tile([P, wmax], F32, tag="bvf")
                nc.sync.dma_start(
                    bvf[:, :kw],
                    bv_d[:][qg:qg + 1, klo:khi].to_broadcast([P, kw]),
                )
                kts = ktsp.tile([P, 2, wmax], MDT, tag="kts")
                for mc in range(2):
                    nc.gpsimd.tensor_tensor(
                        kts[:, mc, :kw], kT_sb[:, mc, klo:khi], bvf[:, :kw],
                        mybir.AluOpType.mult,
                    )
                return kts

            def attn(qg, kts):
                """score/exp/PV chains for all heads; returns ctx psums+dens."""
                klo = kc_lo[qg] * KC
                ctxps = [None] * NH
                dens = densp.tile([NH, QG], F32, tag="dens")
                kcs = list(range(kc_lo[qg], 4 * (qg + 1)))
                for hp in range(2):
                    # two heads of one 128-row kT chunk run CONCURRENTLY on
                    # the PE via row-tiling (array rows 0-63 / 64-127), and
                    # share one 2-bank score tile + one wide exp
                    h0, h1 = 2 * hp, 2 * hp + 1
                    cps_pair = []
                    for h in (h0, h1):
                        cps = cpool.tile([VW, QG], F32, tag="ctx")
                        cps_pair.append(cps)
                    for kc in kcs:
                        q_off = max(0, KC * (kc - 4 * qg))
                        co = kc * KC - klo
                        sp2 = spool.tile([P, 2, QG], F32, tag="spsum")
                        for j, h in enumerate((h0, h1)):
                            pb = (h % 2) * HD
                            nc.tensor.matmul(
                                sp2[:, j, q_off:],
                                lhsT=kts[pb:pb + HD, hp, co:co + KC],
                                rhs=qT_sb[pb:pb + HD, hp,
                                          qg * QG + q_off:(qg + 1) * QG],
                                start=True,
                                stop=True,
                            )
                        if kc >= 4 * qg:  # diagonal: mask both heads' bands
                            band = bass.AP(
                                tensor=sp2.tensor, offset=sp2.offset + q_off,
                                ap=[list(sp2.ap[0]), [QG, 2], [1, KC]],
                            )
                            nc.vector.tensor_tensor(
                                band, band, bm_sb[:, None, :].to_broadcast(
                                    [P, 2, KC]),
                                mybir.AluOpType.add,
                            )
                        pt = ptp.tile([P, 2, QG], MDT, tag="pt")
                        nc.scalar.activation(
                            pt[:, :, q_off:], sp2[:, :, q_off:],
                            mybir.ActivationFunctionType.Exp,
                        )
                        for j, h in enumerate((h0, h1)):
                            nc.tensor.matmul(
                                cps_pair[j][:, q_off:],
                                lhsT=va_sb[:, kc, h * VW:(h + 1) * VW],
                                rhs=pt[:, j, q_off:],
                                start=(kc == kcs[0]),
                                stop=(kc == kcs[-1]),
                            )
                    for j, h in enumerate((h0, h1)):
                        cps = cps_pair[j]
                        # denominator (PSUM partition 64) -> SBUF -> row h
                        d64 = densp.tile([HD + 1, QG], F32, tag="d64")
                        nc.vector.tensor_copy(d64[HD:HD + 1, :],
                                              cps[HD:HD + 1, :])
                        nc.sync.dma_start(dens[h:h + 1, :], d64[HD:HD + 1, :])
                        # undivided ctx to SBUF, freeing the accumulation bank
                        cxf = ctxp.tile([HD, QG], F32, tag="cxf")
                        nc.scalar.copy(cxf, cps[:HD, :])
                        ctxps[h] = cxf
                # add the distant-past count to the denominators
                nc.vector.tensor_scalar_add(dens, dens, cnt_sb[:, qg:qg + 1])
                return ctxps, dens

            def tail(qg, ctxps, dens):
                """reciprocal (via PE transpose + 128-lane DVE), divide ctx,
                out-project, store."""
                dtp = ppool.tile([P, NH * (QG // P)], F32, tag="pp")
                for ss in range(QG // P):
                    nc.tensor.matmul(
                        dtp[:, ss * NH:(ss + 1) * NH],
                        lhsT=dens[:, ss * P:(ss + 1) * P],
                        rhs=eye4_sb,
                        start=True,
                        stop=True,
                    )
                rct = densp.tile([P, NH * (QG // P)], F32, tag="rct")
                nc.vector.reciprocal(rct, dtp)
                rps = ppool.tile([NH, QG], F32, tag="pp")
                for ss in range(QG // P):
                    nc.tensor.matmul(
                        rps[:, ss * P:(ss + 1) * P],
                        lhsT=rct[:, ss * NH:(ss + 1) * NH],
                        rhs=eye128_sb,
                        start=True,
                        stop=True,
                    )
                rec = densp.tile([NH, QG], F32, tag="rec")
                nc.vector.tensor_copy(rec, rps)
                ctx_sb = []
                for h in range(NH):
                    recl = densp.tile([1, QG], F32, tag="recl")
                    nc.sync.dma_start(recl, rec[h:h + 1, :])
                    bcs = densp.tile([HD, QG], F32, tag="bcs")
                    nc.gpsimd.partition_broadcast(bcs, recl)
                    csb = ctxp.tile([HD, QG], MDT, tag="ctxsb")
                    ctx_sb.append(csb)
                    # ctx = (near_ctx + distant_prefix) * (1/den)
                    nc.vector.scalar_tensor_tensor(
                        csb, ctxps[h],
                        pft_sb[:HD, qg * NH + h:qg * NH + h + 1],
                        bcs,
                        mybir.AluOpType.add,
                        mybir.AluOpType.mult,
                    )
                for ss in range(QG // P):
                    ops = ppool.tile([P, DM], F32, tag="pp")
                    for h in range(NH):
                        nc.tensor.matmul(
                            ops,
                            lhsT=ctx_sb[h][:, ss * P:(ss + 1) * P],
                            rhs=wo_sb[:, h, :],
                            start=(h == 0),
                            stop=(h == NH - 1),
                        )
                    osb = ptp.tile([P, DM], F32, tag="osb")
                    nc.vector.tensor_copy(osb, ops)
                    nc.sync.dma_start(
                        out_d[:][qg * QG + ss * P:qg * QG + (ss + 1) * P, :],
                        osb,
                    )

            # two-group lookahead: projections + scaled-k prefetch run well
            # ahead of the attention group that consumes them
            proj(0)
            ktss = [prep(0)]
            proj(1)
            ktss.append(prep(1))
            pending = None
            for qg in range(NQG):
                ctxps, dens = attn(qg, ktss[qg])
                if qg + 2 < NQG:
                    proj(qg + 2)
                    ktss.append(prep(qg + 2))
                if pending is not None:
                    tail(*pending)
                pending = (qg, ctxps, dens)
            tail(*pending)

    nc.finalize()
    return nc


# --------------------------------------------------------------------------
# host wrapper
# --------------------------------------------------------------------------

def _is_tril(mask: np.ndarray) -> bool:
    tril = np.tril(np.ones((S, S), dtype=mask.dtype))
    return all(np.array_equal(mask[b], tril) for b in range(mask.shape[0]))


def _prep_core_inputs(x, days, Wq, bq, Wk, bk, Wv, bv, Wo, rate,
                      use_bf16):
    """Per-core in_maps plus static loop bounds (shared across cores)."""
    t = days.astype(np.float64)  # [B, S]
    # distance beyond which exp(s * decay) == 1.0f exactly: need
    # |s| * exp(-rate*d) < 2^-25 with a generous |s| <= 150 bound.
    d_cut = (np.log(150.0) + 25.5 * np.log(2.0)) / rate
    # static near-window bounds (min over batches so one program fits all)
    kc_lo = []
    for qg in range(NQG):
        lo = NKC
        for b in range(B):
            tq = t[b, qg * QG]
            c = 0
            while c < 4 * qg and t[b, c * KC + KC - 1] < tq - d_cut:
                c += 1
            lo = min(lo, c)
        kc_lo.append(lo)
    kc_lo = tuple(kc_lo)
    wmax = max((qg + 1) * QG - kc_lo[qg] * KC for qg in range(NQG))
    wmax = ((wmax + P - 1) // P) * P

    # per-batch decay factor vectors (f64 for exactness, then f32)
    scale = 1.0 / np.sqrt(HD)
    t0 = np.stack([(t[:, qg * QG] + t[:, qg * QG + QG - 1]) * 0.5
                   for qg in range(NQG)], axis=1)  # [B, NQG]
    avec = np.zeros((B, 1, S), np.float32)
    bvec = np.zeros((B, NQG, S), np.float32)
    for b in range(B):
        for qg in range(NQG):
            sl = slice(qg * QG, (qg + 1) * QG)
            avec[b, 0, sl] = (np.exp(-rate * (t[b, sl] - t0[b, qg])) * scale
                              ).astype(np.float32)
            hi = (qg + 1) * QG
            bvec[b, qg, :hi] = (np.exp(rate * (t[b, :hi] - t0[b, qg]))
                                ).astype(np.float32)
    assert np.all(np.isfinite(avec)) and np.all(np.isfinite(bvec)), \
        "decay factor overflow; q-group span too large for fast path"

    # band mask: keep (0.0) iff q_local >= k_local else -1e30
    kl = np.arange(P)[:, None]
    ql = np.arange(P)[None, :]
    bandm = np.where(ql >= kl, 0.0, NEG).astype(np.float32)

    with_bqk = bool(np.any(bq != 0) or np.any(bk != 0))
    with_bv = bool(np.any(bv != 0))

    in_maps = []
    for c in range(NCORES):
        b, hg = divmod(c, NHG)
        cols = slice(hg * HGD, (hg + 1) * HGD)
        # prefix V sums for the distant rank-1 update: [HD, NQG*NH] (hd-major)
        prefv = np.zeros((HD, NQG * NH), np.float32)
        cnt = np.zeros((NH, NQG), np.float32)
        for qg in range(NQG):
            n = kc_lo[qg] * KC
            cnt[:, qg] = float(n)
            if n > 0:
                xs = x[b, :n].astype(np.float64).sum(axis=0)  # [DM]
                vs = xs @ Wv[cols, :].astype(np.float64).T \
                    + n * bv[cols].astype(np.float64)
                for h in range(NH):
                    prefv[:, qg * NH + h] = \
                        vs[h * HD:(h + 1) * HD].astype(np.float32)
        mdt = np.dtype(ml_dtypes.bfloat16) if use_bf16 else np.float32
        m = {
            "xT": np.ascontiguousarray(x[b].T).astype(mdt),
            "wqT": np.ascontiguousarray(Wq[cols, :].T).astype(mdt),
            "wkT": np.ascontiguousarray(Wk[cols, :].T).astype(mdt),
            "wvT": np.ascontiguousarray(Wv[cols, :].T).astype(mdt),
            "woT": np.ascontiguousarray(Wo[:, cols].T).astype(mdt),
            "avec": avec[b],
            "bvec": bvec[b],
            "prefv": prefv,
            "cnt": cnt,
            "bandm": bandm,
        }
        if with_bqk:
            m["bq"] = np.ascontiguousarray(
                bq[cols].reshape(2, P).T).astype(np.float32)
            m["bk"] = np.ascontiguousarray(
                bk[cols].reshape(2, P).T).astype(np.float32)
        if with_bv:
            m["bvb"] = bv[cols].reshape(1, HGD).astype(np.float32)
        in_maps.append(m)
    return in_maps, kc_lo, wmax, with_bqk, with_bv


def _reference_host(x, mask, days_offset, Wq, bq, Wk, bk, Wv, bv, Wo, bo,
                    decay_rate):
    """Emergency numpy fallback for inputs outside the fast path."""
    b, s, _ = x.shape
    out = np.empty((b, s, DM), np.float32)
    for bi in range(b):
        q = (x[bi] @ Wq.T + bq).reshape(s, H, HD).transpose(1, 0, 2)
        k = (x[bi] @ Wk.T + bk).reshape(s, H, HD).transpose(1, 0, 2)
        v = (x[bi] @ Wv.T + bv).reshape(s, H, HD).transpose(1, 0, 2)
        dist = np.abs(days_offset[bi][:, None] - days_offset[bi][None, :])
        decay = np.exp(-decay_rate * dist).astype(np.float32)
        ctx = np.empty((H, s, HD), np.float32)
        for h in range(H):
            sc = (q[h] @ k[h].T) / np.sqrt(HD) * decay
            sc = np.where(mask[bi] == 0, -np.inf, sc)
            sc = sc - sc.max(axis=-1, keepdims=True)
            e = np.exp(sc)
            ctx[h] = (e / e.sum(axis=-1, keepdims=True)) @ v[h]
        out[bi] = ctx.transpose(1, 0, 2).reshape(s, DM) @ Wo.T + bo
    return out


def kernel(x, mask, days_offset, Wq, bq, Wk, bk, Wv, bv, Wo, bo, decay_rate,
           _trace=False):
    x = np.asarray(x, np.float32)
    mask = np.asarray(mask)
    days = np.asarray(days_offset, np.float32)
    Wq, bq = np.asarray(Wq, np.float32), np.asarray(bq, np.float32)
    Wk, bk = np.asarray(Wk, np.float32), np.asarray(bk, np.float32)
    Wv, bv = np.asarray(Wv, np.float32), np.asarray(bv, np.float32)
    Wo, bo = np.asarray(Wo, np.float32), np.asarray(bo, np.float32)
    rate = float(np.asarray(decay_rate))

    sorted_ok = bool(np.all(np.diff(days, axis=-1) >= 0))
    if not (sorted_ok and _is_tril(mask)):
        return _reference_host(x, mask, days, Wq, bq, Wk, bk, Wv, bv, Wo, bo,
                               rate)

    use_bf16 = os.environ.get("KERNEL_F32", "") != "1"
    in_maps, kc_lo, wmax, with_bqk, with_bv = _prep_core_inputs(
        x, days, Wq, bq, Wk, bk, Wv, bv, Wo, rate, use_bf16)

    key = (kc_lo, wmax, with_bqk, with_bv, use_bf16)
    if key not in _cache:
        _cache[key] = _build_fast(kc_lo, wmax, with_bqk, with_bv, use_bf16)
    nc = _cache[key]

    res = run_bass_kernel_spmd(nc, in_maps, core_ids=list(range(NCORES)),
                               trace=_trace)
    out = np.empty((B, S, DM), np.float32)
    for b in range(B):
        out[b] = res.results[2 * b]["outp"] + res.results[2 * b + 1]["outp"] + bo
    if _trace:
        return out, res
    return out


# revision 41
# speedup vs baseline: 1.1719x; 1.1201x over previous
"""Trainium2 Bass kernel for temporal-decay causal multi-head attention.

Problem: nn_MultiHeadAttention_9053791060340
  B=4, S=2048, DM=512, H=8, HD=64.
  out = softmax((Q K^T / sqrt(HD)) * exp(-rate*|t_i - t_j|) with causal mask) V,
  then out-projection.

Sharding: 8 cores = 4 batches x 2 head-groups (4 heads each). Each core
computes a partial out-projection [S, DM] for its head group; the host sums
the two partials per batch and adds the output bias.

Device algorithm (per core); matmul inputs in bf16 with fp32 PSUM
accumulation by default (set KERNEL_F32=1 for all-fp32, ~2.2x slower,
error ~1e-6 instead of ~3e-3 scale-relative):
  - scores are computed TRANSPOSED (S^T[k, q] = k . q), so softmax-free-axis
    tricks are unnecessary: we use a no-max softmax (scores here are bounded,
    |v| <~ 64, so exp never overflows in fp32), and the denominator comes for
    free from a ones-column appended to V (PV matmul directly accumulates
    ctx^T[hd, q] plus the row of denominators).
  - temporal decay factorizes on sorted days: exp(-r(t_i - t_k)) = a_i * b_k
    with a per-q-group reference t0 to keep the factors in fp32 range.
    a (and the 1/sqrt(HD) scale) is folded into q^T once; b is folded into a
    per-q-group scaled copy of k^T.
  - pairs far enough apart (rate*dist > ~26) have s*decay so small that
    exp(s*decay) == 1.0f exactly (reference behaves identically), so all
    k-chunks entirely below the cutoff collapse into a rank-1 update
    ctx^T += PrefVsum x ones, with PrefVsum precomputed on the host.
  - causal masking needs work only in the diagonal 128x128 band of each
    q-group: an additive -1e30 tril tile is applied to S^T in PSUM before the
    exp. Blocks fully above the diagonal are skipped by restricting the
    streamed q-range.
"""

import os

import ml_dtypes
import numpy as np

import concourse.bass as bass
import concourse.tile as tile
from concourse import bacc
from concourse import mybir
from concourse.bass_utils import run_bass_kernel_spmd
from concourse.masks import make_identity

F32 = mybir.dt.float32

B, S, DM, H = 4, 2048, 512, 8
HD = DM // H          # 64
NCORES = 8
NHG = 2               # head groups == cores per batch
NH = H // NHG         # heads per core
HGD = NH * HD         # 256 output dims per core
QG = 512              # q-group width
NQG = S // QG         # 4
KC = 128              # k chunk (partition dim of S^T)
NKC = S // KC         # 16
P = 128
NEG = -1.0e30

_cache: dict = {}


# --------------------------------------------------------------------------
# device program
# --------------------------------------------------------------------------

def _build_fast(kc_lo: tuple, wmax: int, with_bqk: bool, with_bv: bool,
                use_bf16: bool = True):
    """Build the SPMD Bass program.

    kc_lo[qg] = first near k-chunk per q-group (static across cores; chunks
    below it are covered by the prefix-sum rank-1 update). wmax = max
    near-window width in elements for the scaled-k tile. use_bf16 casts all
    matmul inputs to bf16 (fp32 PSUM accumulation): the PE streams 1 col/cyc
    for bf16 vs 2 for fp32, halving matmul time.

    The q-group loop is software-pipelined: prep(qg+1) (b-vector DMA +
    scaled-k) is emitted before tail(qg) (reciprocal, divide, out-project)
    so the PE always has next-group score matmuls available while the
    denominator pipeline of the previous group drains.
    """
    nc = bacc.Bacc()
    MDT = mybir.dt.bfloat16 if use_bf16 else F32

    xT_d = nc.declare_dram_parameter("xT", [DM, S], MDT, False)
    wq_d = nc.declare_dram_parameter("wqT", [DM, HGD], MDT, False)
    wk_d = nc.declare_dram_parameter("wkT", [DM, HGD], MDT, False)
    wv_d = nc.declare_dram_parameter("wvT", [DM, HGD], MDT, False)
    wo_d = nc.declare_dram_parameter("woT", [HGD, DM], MDT, False)
    av_d = nc.declare_dram_parameter("avec", [1, S], F32, False)
    bv_d = nc.declare_dram_parameter("bvec", [NQG, S], F32, False)
    pf_d = nc.declare_dram_parameter("prefv", [HD, NQG * NH], F32, False)
    ct_d = nc.declare_dram_parameter("cnt", [NH, NQG], F32, False)
    bm_d = nc.declare_dram_parameter("bandm", [P, P], F32, False)
    if with_bqk:
        bq_d = nc.declare_dram_parameter("bq", [P, 2], F32, False)
        bk_d = nc.declare_dram_parameter("bk", [P, 2], F32, False)
    if with_bv:
        bvb_d = nc.declare_dram_parameter("bvb", [1, HGD], F32, False)
    out_d = nc.declare_dram_parameter("outp", [S, DM], F32, True)

    KO = DM // P  # 4 k-sub-chunks for DM-contraction
    VW = HD + 1   # 65: V columns plus ones column

    with tile.TileContext(nc) as tc:
        with (
            tc.tile_pool(name="const", bufs=1) as const,
            tc.tile_pool(name="ppool", bufs=2, space="PSUM") as ppool,
            tc.tile_pool(name="spool", bufs=2, space="PSUM") as spool,
            tc.tile_pool(name="cpool", bufs=2, space="PSUM") as cpool,
            tc.tile_pool(name="ptp", bufs=3) as ptp,
            tc.tile_pool(name="ktsp", bufs=3) as ktsp,
            tc.tile_pool(name="bvqp", bufs=3) as bvqp,
            tc.tile_pool(name="ctxp", bufs=6) as ctxp,
            tc.tile_pool(name="densp", bufs=2) as densp,
        ):
            # ---- constant loads (weights first; x chunked by seq slice) ----
            wq_sb = const.tile([P, KO, HGD], MDT)
            nc.sync.dma_start(wq_sb, wq_d[:].rearrange("(ko p) m -> p ko m", p=P))
            wk_sb = const.tile([P, KO, HGD], MDT)
            nc.sync.dma_start(wk_sb, wk_d[:].rearrange("(ko p) m -> p ko m", p=P))
            wv_sb = const.tile([P, KO, HGD], MDT)
            nc.sync.dma_start(wv_sb, wv_d[:].rearrange("(ko p) m -> p ko m", p=P))
            xT_sb = const.tile([P, KO, S], MDT)
            xT_r = xT_d[:].rearrange("(ko p) s -> p ko s", p=P)
            for ns in range(4):
                nc.sync.dma_start(xT_sb[:, :, ns * QG:(ns + 1) * QG],
                                  xT_r[:, :, ns * QG:(ns + 1) * QG])
            # head-pair on partitions: rows 0-63 = even head, 64-127 = odd
            wo_sb = const.tile([P, 2, DM], MDT)
            nc.sync.dma_start(wo_sb, wo_d[:].rearrange("(hp p) n -> p hp n", p=P))

            avec_full = const.tile([P, S], F32)
            nc.sync.dma_start(avec_full, av_d[:].to_broadcast([P, S]))
            pft_sb = const.tile([HD, NQG * NH], F32)
            nc.sync.dma_start(pft_sb, pf_d[:])
            cnt_sb = const.tile([NH, NQG], F32)
            nc.sync.dma_start(cnt_sb, ct_d[:])
            bm_sb = const.tile([P, P], F32)
            nc.sync.dma_start(bm_sb, bm_d[:])
            eye4_sb = const.tile([NH, NH], F32)
            make_identity(nc, eye4_sb)
            eye128_sb = const.tile([P, P], F32)
            make_identity(nc, eye128_sb)
            if with_bqk:
                bq_sb = const.tile([P, 2], F32)
                nc.sync.dma_start(bq_sb, bq_d[:])
                bk_sb = const.tile([P, 2], F32)
                nc.sync.dma_start(bk_sb, bk_d[:])
            if with_bv:
                bv_full = const.tile([P, HGD], F32)
                nc.sync.dma_start(bv_full, bvb_d[:].to_broadcast([P, HGD]))

            # ---- projections (emitted per q-group, interleaved with the
            # ACT-bound attention stream so the PE stays dense) ----
            qT_f32 = const.tile([P, 2, S], F32)
            kT_sb = const.tile([P, 2, S], F32)
            qT_sb = const.tile([P, 2, S], MDT, name='qT_cast') if use_bf16 else qT_f32
            va_sb = const.tile([P, NKC, NH * VW], MDT)
            va_resh = va_sb.rearrange("p s (h c) -> p s h c", c=VW)
            nc.vector.memset(va_resh[:, :, :, HD], 1.0)

            def proj(ns):
                """q/k/v projections for sequence slice ns (one q-group)."""
                sl = slice(ns * QG, (ns + 1) * QG)
                for w_sb, t_sb, b_sb in (
                    (wq_sb, qT_f32, "q"),
                    (wk_sb, kT_sb, "k"),
                ):
                    for mc in range(2):
                        ps = ppool.tile([P, QG], F32, tag="pp")
                        for ki in range(KO):
                            nc.tensor.matmul(
                                ps,
                                lhsT=w_sb[:, ki, mc * P:(mc + 1) * P],
                                rhs=xT_sb[:, ki, sl],
                                start=(ki == 0),
                                stop=(ki == KO - 1),
                            )
                        if with_bqk:
                            bias = (bq_sb if b_sb == "q" else bk_sb)[:, mc:mc + 1]
                            nc.scalar.activation(
                                t_sb[:, mc, sl], ps,
                                mybir.ActivationFunctionType.Identity,
                                bias=bias,
                            )
                        else:
                            nc.scalar.copy(t_sb[:, mc, sl], ps)
                # fold a (and 1/sqrt(HD)) into q^T on the idle GPSIMD
                for mc in range(2):
                    nc.gpsimd.tensor_tensor(
                        qT_sb[:, mc, sl], qT_f32[:, mc, sl],
                        avec_full[:, sl], mybir.AluOpType.mult,
                    )
                for sc in range(4 * ns, 4 * ns + 4):
                    ps = ppool.tile([P, HGD], F32, tag="pp")
                    for ki in range(KO):
                        nc.tensor.matmul(
                            ps,
                            lhsT=xT_sb[:, ki, sc * P:(sc + 1) * P],
                            rhs=wv_sb[:, ki, :],
                            start=(ki == 0),
                            stop=(ki == KO - 1),
                        )
                    for h in range(NH):
                        dst = va_sb[:, sc, h * VW:h * VW + HD]
                        src = ps[:, h * HD:(h + 1) * HD]
                        if with_bv:
                            nc.vector.tensor_tensor(
                                dst, src, bv_full[:, h * HD:(h + 1) * HD],
                                mybir.AluOpType.add,
                            )
                        else:
                            nc.vector.tensor_copy(dst, src)

            # ---- attention + out-projection: software-pipelined q-groups --
            def prep(qg):
                """b-vector broadcast DMA + b-scaled k^T for group qg."""
                klo = kc_lo[qg] * KC
                khi = (qg + 1) * QG
                kw = khi - klo
                bvf = bvqp.tile([P, wmax], F32, tag="bvf")
                nc.sync.dma_start(
                    bvf[:, :kw],
                    bv_d[:][qg:qg + 1, klo:khi].to_broadcast([P, kw]),
                )
                kts = ktsp.tile([P, 2, wmax], MDT, tag="kts")
                for mc in range(2):
                    nc.gpsimd.tensor_tensor(
                        kts[:, mc, :kw], kT_sb[:, mc, klo:khi], bvf[:, :kw],
                        mybir.AluOpType.mult,
                    )
                return kts

            def attn(qg, kts):
                """score/exp/PV chains for all heads; returns ctx psums+dens."""
                klo = kc_lo[qg] * KC
                ctxps = [None] * NH
                dens = densp.tile([NH, QG], F32, tag="dens")
                kcs = list(range(kc_lo[qg], 4 * (qg + 1)))
                for hp in range(2):
                    # two heads of one 128-row kT chunk run CONCURRENTLY on
                    # the PE via row-tiling (array rows 0-63 / 64-127), and
                    # share one 2-bank score tile + one wide exp
                    h0, h1 = 2 * hp, 2 * hp + 1
                    cps_pair = []
                    for h in (h0, h1):
                        cps = cpool.tile([VW, QG], F32, tag="ctx")
                        cps_pair.append(cps)
                    for kc in kcs:
                        q_off = max(0, KC * (kc - 4 * qg))
                        co = kc * KC - klo
                        sp2 = spool.tile([P, 2, QG], F32, tag="spsum")
                        for j, h in enumerate((h0, h1)):
                            pb = (h % 2) * HD
                            nc.tensor.matmul(
                                sp2[:, j, q_off:],
                                lhsT=kts[pb:pb + HD, hp, co:co + KC],
                                rhs=qT_sb[pb:pb + HD, hp,
                                          qg * QG + q_off:(qg + 1) * QG],
                                start=True,
                                stop=True,
                            )
                        if kc >= 4 * qg:  # diagonal: mask both heads' bands
                            band = bass.AP(
                                tensor=sp2.tensor, offset=sp2.offset + q_off,
                                ap=[list(sp2.ap[0]), [QG, 2], [1, KC]],
                            )
                            nc.vector.tensor_tensor(
                                band, band, bm_sb[:, None, :].to_broadcast(
                                    [P, 2, KC]),
                                mybir.AluOpType.add,
                            )
                        pt = ptp.tile([P, 2, QG], MDT, tag="pt")
                        nc.scalar.activation(
                            pt[:, :, q_off:], sp2[:, :, q_off:],
                            mybir.ActivationFunctionType.Exp,
                        )
                        for j, h in enumerate((h0, h1)):
                            nc.tensor.matmul(
                                cps_pair[j][:, q_off:],
                                lhsT=va_sb[:, kc, h * VW:(h + 1) * VW],
                                rhs=pt[:, j, q_off:],
                                start=(kc == kcs[0]),
                                stop=(kc == kcs[-1]),
                            )
                    for j, h in enumerate((h0, h1)):
                        cps = cps_pair[j]
                        # denominator (PSUM partition 64) -> SBUF -> row h
                        d64 = densp.tile([HD + 1, QG], F32, tag="d64")
                        nc.vector.tensor_copy(d64[HD:HD + 1, :],
                                              cps[HD:HD + 1, :])
                        nc.sync.dma_start(dens[h:h + 1, :], d64[HD:HD + 1, :])
                        # undivided ctx to SBUF, freeing the accumulation bank
                        cxf = ctxp.tile([HD, QG], F32, tag="cxf")
                        nc.scalar.copy(cxf, cps[:HD, :])
                        ctxps[h] = cxf
                # add the distant-past count to the denominators
                nc.vector.tensor_scalar_add(dens, dens, cnt_sb[:, qg:qg + 1])
                return ctxps, dens

            def tail(qg, ctxps, dens):
                """reciprocal (via PE transpose + 128-lane DVE), divide ctx,
                out-project, store."""
                dtp = ppool.tile([P, NH * (QG // P)], F32, tag="pp")
                for ss in range(QG // P):
                    nc.tensor.matmul(
                        dtp[:, ss * NH:(ss + 1) * NH],
                        lhsT=dens[:, ss * P:(ss + 1) * P],
                        rhs=eye4_sb,
                        start=True,
                        stop=True,
                    )
                rct = densp.tile([P, NH * (QG // P)], F32, tag="rct")
                nc.vector.reciprocal(rct, dtp)
                rps = ppool.tile([NH, QG], F32, tag="pp")
                for ss in range(QG // P):
                    nc.tensor.matmul(
                        rps[:, ss * P:(ss + 1) * P],
                        lhsT=rct[:, ss * NH:(ss + 1) * NH],
                        rhs=eye128_sb,
                        start=True,
                        stop=True,
                    )
                rec = densp.tile([NH, QG], F32, tag="rec")
                nc.vector.tensor_copy(rec, rps)
                ctx_sb = []
                for h in range(NH):
                    recl = densp.tile([1, QG], F32, tag="recl")
                    nc.sync.dma_start(recl, rec[h:h + 1, :])
                    bcs = densp.tile([HD, QG], F32, tag="bcs")
                    nc.gpsimd.partition_broadcast(bcs, recl)
                    csb = ctxp.tile([HD, QG], MDT, tag="ctxsb")
                    ctx_sb.append(csb)
                    # ctx = (near_ctx + distant_prefix) * (1/den)
                    nc.vector.scalar_tensor_tensor(
                        csb, ctxps[h],
                        pft_sb[:HD, qg * NH + h:qg * NH + h + 1],
                        bcs,
                        mybir.AluOpType.add,
                        mybir.AluOpType.mult,
                    )
                # repack head pairs onto full 128-partition tiles (DMA is
                # the only cross-partition mover) so the out-projection runs
                # at K=128 instead of two half-array K=64 matmuls
                pairs = []
                for hp in range(2):
                    cp2 = ctxp.tile([P, QG], MDT, tag="cpair")
                    nc.sync.dma_start(cp2[0:HD, :], ctx_sb[2 * hp])
                    nc.sync.dma_start(cp2[HD:P, :], ctx_sb[2 * hp + 1])
                    pairs.append(cp2)
                for ss in range(QG // P):
                    ops = ppool.tile([P, DM], F32, tag="pp")
                    for hp in range(2):
                        nc.tensor.matmul(
                            ops,
                            lhsT=pairs[hp][:, ss * P:(ss + 1) * P],
                            rhs=wo_sb[:, hp, :],
                            start=(hp == 0),
                            stop=(hp == 1),
                        )
                    osb = ptp.tile([P, DM], F32, tag="osb")
                    nc.vector.tensor_copy(osb, ops)
                    nc.sync.dma_start(
                        out_d[:][qg * QG + ss * P:qg * QG + (ss + 1) * P, :],
                        osb,
                    )

            # two-group lookahead: projections + scaled-k prefetch run well
            # ahead of the attention group that consumes them
            proj(0)
            ktss = [prep(0)]
            proj(1)
            ktss.append(prep(1))
            pending = None
            for qg in range(NQG):
                ctxps, dens = attn(qg, ktss[qg])
                if qg + 2 < NQG:
                    proj(qg + 2)
                    ktss.append(prep(qg + 2))
                if pending is not None:
                    tail(*pending)
                pending = (qg, ctxps, dens)
            tail(*pending)

    nc.finalize()
    return nc


# --------------------------------------------------------------------------
# host wrapper
# --------------------------------------------------------------------------

def _is_tril(mask: np.ndarray) -> bool:
    tril = np.tril(np.ones((S, S), dtype=mask.dtype))
    return all(np.array_equal(mask[b], tril) for b in range(mask.shape[0]))


def _prep_core_inputs(x, days, Wq, bq, Wk, bk, Wv, bv, Wo, rate,
                      use_bf16):
    """Per-core in_maps plus static loop bounds (shared across cores)."""
    t = days.astype(np.float64)  # [B, S]
    # distance beyond which exp(s * decay) == 1.0f exactly: need
    # |s| * exp(-rate*d) < 2^-25 with a generous |s| <= 150 bound.
    d_cut = (np.log(150.0) + 25.5 * np.log(2.0)) / rate
    # static near-window bounds (min over batches so one program fits all)
    kc_lo = []
    for qg in range(NQG):
        lo = NKC
        for b in range(B):
            tq = t[b, qg * QG]
            c = 0
            while c < 4 * qg and t[b, c * KC + KC - 1] < tq - d_cut:
                c += 1
            lo = min(lo, c)
        kc_lo.append(lo)
    kc_lo = tuple(kc_lo)
    wmax = max((qg + 1) * QG - kc_lo[qg] * KC for qg in range(NQG))
    wmax = ((wmax + P - 1) // P) * P

    # per-batch decay factor vectors (f64 for exactness, then f32)
    scale = 1.0 / np.sqrt(HD)
    t0 = np.stack([(t[:, qg * QG] + t[:, qg * QG + QG - 1]) * 0.5
                   for qg in range(NQG)], axis=1)  # [B, NQG]
    avec = np.zeros((B, 1, S), np.float32)
    bvec = np.zeros((B, NQG, S), np.float32)
    for b in range(B):
        for qg in range(NQG):
            sl = slice(qg * QG, (qg + 1) * QG)
            avec[b, 0, sl] = (np.exp(-rate * (t[b, sl] - t0[b, qg])) * scale
                              ).astype(np.float32)
            hi = (qg + 1) * QG
            bvec[b, qg, :hi] = (np.exp(rate * (t[b, :hi] - t0[b, qg]))
                                ).astype(np.float32)
    assert np.all(np.isfinite(avec)) and np.all(np.isfinite(bvec)), \
        "decay factor overflow; q-group span too large for fast path"

    # band mask: keep (0.0) iff q_local >= k_local else -1e30
    kl = np.arange(P)[:, None]
    ql = np.arange(P)[None, :]
    bandm = np.where(ql >= kl, 0.0, NEG).astype(np.float32)

    with_bqk = bool(np.any(bq != 0) or np.any(bk != 0))
    with_bv = bool(np.any(bv != 0))

    in_maps = []
    for c in range(NCORES):
        b, hg = divmod(c, NHG)
        cols = slice(hg * HGD, (hg + 1) * HGD)
        # prefix V sums for the distant rank-1 update: [HD, NQG*NH] (hd-major)
        prefv = np.zeros((HD, NQG * NH), np.float32)
        cnt = np.zeros((NH, NQG), np.float32)
        for qg in range(NQG):
            n = kc_lo[qg] * KC
            cnt[:, qg] = float(n)
            if n > 0:
                xs = x[b, :n].astype(np.float64).sum(axis=0)  # [DM]
                vs = xs @ Wv[cols, :].astype(np.float64).T \
                    + n * bv[cols].astype(np.float64)
                for h in range(NH):
                    prefv[:, qg * NH + h] = \
                        vs[h * HD:(h + 1) * HD].astype(np.float32)
        mdt = np.dtype(ml_dtypes.bfloat16) if use_bf16 else np.float32
        m = {
            "xT": np.ascontiguousarray(x[b].T).astype(mdt),
            "wqT": np.ascontiguousarray(Wq[cols, :].T).astype(mdt),
            "wkT": np.ascontiguousarray(Wk[cols, :].T).astype(mdt),
            "wvT": np.ascontiguousarray(Wv[cols, :].T).astype(mdt),
            "woT": np.ascontiguousarray(Wo[:, cols].T).astype(mdt),
            "avec": avec[b],
            "bvec": bvec[b],
            "prefv": prefv,
            "cnt": cnt,
            "bandm": bandm,
        }
        if with_bqk:
            m["bq"] = np.ascontiguousarray(
                bq[cols].reshape(2, P).T).astype(np.float32)
            m["bk"] = np.ascontiguousarray(
                bk[cols].reshape(2, P).T).astype(np.float32)
        if with_bv:
            m["bvb"] = bv[cols].reshape(1, HGD).astype(np.float32)
        in_maps.append(m)
    return in_maps, kc_lo, wmax, with_bqk, with_bv


def _reference_host(x, mask, days_offset, Wq, bq, Wk, bk, Wv, bv, Wo, bo,
                    decay_rate):
    """Emergency numpy fallback for inputs outside the fast path."""
    b, s, _ = x.shape
    out = np.empty((b, s, DM), np.float32)
    for bi in range(b):
        q = (x[bi] @ Wq.T + bq).reshape(s, H, HD).transpose(1, 0, 2)
        k = (x[bi] @ Wk.T + bk).reshape(s, H, HD).transpose(1, 0, 2)
        v = (x[bi] @ Wv.T + bv).reshape(s, H, HD).transpose(1, 0, 2)
        dist = np.abs(days_offset[bi][:, None] - days_offset[bi][None, :])
        decay = np.exp(-decay_rate * dist).astype(np.float32)
        ctx = np.empty((H, s, HD), np.float32)
        for h in range(H):
            sc = (q[h] @ k[h].T) / np.sqrt(HD) * decay
            sc = np.where(mask[bi] == 0, -np.inf, sc)
            sc = sc - sc.max(axis=-1, keepdims=True)
            e = np.exp(sc)
            ctx[h] = (e / e.sum(axis=-1, keepdims=True)) @ v[h]
        out[bi] = ctx.transpose(1, 0, 2).reshape(s, DM) @ Wo.T + bo
    return out


def kernel(x, mask, days_offset, Wq, bq, Wk, bk, Wv, bv, Wo, bo, decay_rate,
           _trace=False):
    x = np.asarray(x, np.float32)
    mask = np.asarray(mask)
    days = np.asarray(days_offset, np.float32)
    Wq, bq = np.asarray(Wq, np.float32), np.asarray(bq, np.float32)
    Wk, bk = np.asarray(Wk, np.float32), np.asarray(bk, np.float32)
    Wv, bv = np.asarray(Wv, np.float32), np.asarray(bv, np.float32)
    Wo, bo = np.asarray(Wo, np.float32), np.asarray(bo, np.float32)
    rate = float(np.asarray(decay_rate))

    sorted_ok = bool(np.all(np.diff(days, axis=-1) >= 0))
    if not (sorted_ok and _is_tril(mask)):
        return _reference_host(x, mask, days, Wq, bq, Wk, bk, Wv, bv, Wo, bo,
                               rate)

    use_bf16 = os.environ.get("KERNEL_F32", "") != "1"
    in_maps, kc_lo, wmax, with_bqk, with_bv = _prep_core_inputs(
        x, days, Wq, bq, Wk, bk, Wv, bv, Wo, rate, use_bf16)

    key = (kc_lo, wmax, with_bqk, with_bv, use_bf16)
    if key not in _cache:
        _cache[key] = _build_fast(kc_lo, wmax, with_bqk, with_bv, use_bf16)
    nc = _cache[key]

    res = run_bass_kernel_spmd(nc, in_maps, core_ids=list(range(NCORES)),
                               trace=_trace)
    out = np.empty((B, S, DM), np.float32)
    for b in range(B):
        out[b] = res.results[2 * b]["outp"] + res.results[2 * b + 1]["outp"] + bo
    if _trace:
        return out, res
    return out


# revision 42
# speedup vs baseline: 1.2037x; 1.0271x over previous
"""Trainium2 Bass kernel for temporal-decay causal multi-head attention.

Problem: nn_MultiHeadAttention_9053791060340
  B=4, S=2048, DM=512, H=8, HD=64.
  out = softmax((Q K^T / sqrt(HD)) * exp(-rate*|t_i - t_j|) with causal mask) V,
  then out-projection.

Sharding: 8 cores = 4 batches x 2 head-groups (4 heads each). Each core
computes a partial out-projection [S, DM] for its head group; the host sums
the two partials per batch and adds the output bias.

Device algorithm (per core); matmul inputs in bf16 with fp32 PSUM
accumulation by default (set KERNEL_F32=1 for all-fp32, ~2.2x slower,
error ~1e-6 instead of ~3e-3 scale-relative):
  - scores are computed TRANSPOSED (S^T[k, q] = k . q), so softmax-free-axis
    tricks are unnecessary: we use a no-max softmax (scores here are bounded,
    |v| <~ 64, so exp never overflows in fp32), and the denominator comes for
    free from a ones-column appended to V (PV matmul directly accumulates
    ctx^T[hd, q] plus the row of denominators).
  - temporal decay factorizes on sorted days: exp(-r(t_i - t_k)) = a_i * b_k
    with a per-q-group reference t0 to keep the factors in fp32 range.
    a (and the 1/sqrt(HD) scale) is folded into q^T once; b is folded into a
    per-q-group scaled copy of k^T.
  - pairs far enough apart (rate*dist > ~26) have s*decay so small that
    exp(s*decay) == 1.0f exactly (reference behaves identically), so all
    k-chunks entirely below the cutoff collapse into a rank-1 update
    ctx^T += PrefVsum x ones, with PrefVsum precomputed on the host.
  - causal masking needs work only in the diagonal 128x128 band of each
    q-group: an additive -1e30 tril tile is applied to S^T in PSUM before the
    exp. Blocks fully above the diagonal are skipped by restricting the
    streamed q-range.
"""

import os

import ml_dtypes
import numpy as np

import concourse.bass as bass
import concourse.tile as tile
from concourse import bacc
from concourse import mybir
from concourse.bass_utils import run_bass_kernel_spmd
from concourse.masks import make_identity

F32 = mybir.dt.float32

B, S, DM, H = 4, 2048, 512, 8
HD = DM // H          # 64
NCORES = 8
NHG = 2               # head groups == cores per batch
NH = H // NHG         # heads per core
HGD = NH * HD         # 256 output dims per core
QG = 512              # q-group width
NQG = S // QG         # 4
KC = 128              # k chunk (partition dim of S^T)
NKC = S // KC         # 16
P = 128
NEG = -1.0e30

_cache: dict = {}


# --------------------------------------------------------------------------
# device program
# --------------------------------------------------------------------------

def _build_fast(kc_lo: tuple, wmax: int, with_bqk: bool, with_bv: bool,
                use_bf16: bool = True):
    """Build the SPMD Bass program.

    kc_lo[qg] = first near k-chunk per q-group (static across cores; chunks
    below it are covered by the prefix-sum rank-1 update). wmax = max
    near-window width in elements for the scaled-k tile. use_bf16 casts all
    matmul inputs to bf16 (fp32 PSUM accumulation): the PE streams 1 col/cyc
    for bf16 vs 2 for fp32, halving matmul time.

    The q-group loop is software-pipelined: prep(qg+1) (b-vector DMA +
    scaled-k) is emitted before tail(qg) (reciprocal, divide, out-project)
    so the PE always has next-group score matmuls available while the
    denominator pipeline of the previous group drains.
    """
    nc = bacc.Bacc()
    MDT = mybir.dt.bfloat16 if use_bf16 else F32

    xT_d = nc.declare_dram_parameter("xT", [DM, S], MDT, False)
    wq_d = nc.declare_dram_parameter("wqT", [DM, HGD], MDT, False)
    wk_d = nc.declare_dram_parameter("wkT", [DM, HGD], MDT, False)
    wv_d = nc.declare_dram_parameter("wvT", [DM, HGD], MDT, False)
    wo_d = nc.declare_dram_parameter("woT", [HGD, DM], MDT, False)
    av_d = nc.declare_dram_parameter("avec", [1, S], F32, False)
    bv_d = nc.declare_dram_parameter("bvec", [NQG, S], F32, False)
    pf_d = nc.declare_dram_parameter("prefv", [HD, NQG * NH], F32, False)
    ct_d = nc.declare_dram_parameter("cnt", [NH, NQG], F32, False)
    bm_d = nc.declare_dram_parameter("bandm", [P, P], F32, False)
    if with_bqk:
        bq_d = nc.declare_dram_parameter("bq", [P, 2], F32, False)
        bk_d = nc.declare_dram_parameter("bk", [P, 2], F32, False)
    if with_bv:
        bvb_d = nc.declare_dram_parameter("bvb", [1, HGD], F32, False)
    out_d = nc.declare_dram_parameter("outp", [S, DM], F32, True)

    KO = DM // P  # 4 k-sub-chunks for DM-contraction
    VW = HD + 1   # 65: V columns plus ones column

    with tile.TileContext(nc) as tc:
        with (
            tc.tile_pool(name="const", bufs=1) as const,
            tc.tile_pool(name="ppool", bufs=2, space="PSUM") as ppool,
            tc.tile_pool(name="spool", bufs=2, space="PSUM") as spool,
            tc.tile_pool(name="cpool", bufs=2, space="PSUM") as cpool,
            tc.tile_pool(name="ptp", bufs=3) as ptp,
            tc.tile_pool(name="ktsp", bufs=3) as ktsp,
            tc.tile_pool(name="bvqp", bufs=3) as bvqp,
            tc.tile_pool(name="ctxp", bufs=6) as ctxp,
            tc.tile_pool(name="densp", bufs=2) as densp,
        ):
            # ---- constant loads (weights first; x chunked by seq slice) ----
            wq_sb = const.tile([P, KO, HGD], MDT)
            nc.sync.dma_start(wq_sb, wq_d[:].rearrange("(ko p) m -> p ko m", p=P))
            wk_sb = const.tile([P, KO, HGD], MDT)
            nc.sync.dma_start(wk_sb, wk_d[:].rearrange("(ko p) m -> p ko m", p=P))
            wv_sb = const.tile([P, KO, HGD], MDT)
            nc.sync.dma_start(wv_sb, wv_d[:].rearrange("(ko p) m -> p ko m", p=P))
            xT_sb = const.tile([P, KO, S], MDT)
            xT_r = xT_d[:].rearrange("(ko p) s -> p ko s", p=P)
            for ns in range(4):
                nc.sync.dma_start(xT_sb[:, :, ns * QG:(ns + 1) * QG],
                                  xT_r[:, :, ns * QG:(ns + 1) * QG])
            # head-pair on partitions: rows 0-63 = even head, 64-127 = odd
            wo_sb = const.tile([P, 2, DM], MDT)
            nc.sync.dma_start(wo_sb, wo_d[:].rearrange("(hp p) n -> p hp n", p=P))

            avec_full = const.tile([P, S], F32)
            nc.sync.dma_start(avec_full, av_d[:].to_broadcast([P, S]))
            pft_sb = const.tile([HD, NQG * NH], F32)
            nc.sync.dma_start(pft_sb, pf_d[:])
            cnt_sb = const.tile([NH, NQG], F32)
            nc.sync.dma_start(cnt_sb, ct_d[:])
            bm_sb = const.tile([P, P], F32)
            nc.sync.dma_start(bm_sb, bm_d[:])
            eye4_sb = const.tile([NH, NH], F32)
            make_identity(nc, eye4_sb)
            eye128_sb = const.tile([P, P], F32)
            make_identity(nc, eye128_sb)
            if with_bqk:
                bq_sb = const.tile([P, 2], F32)
                nc.sync.dma_start(bq_sb, bq_d[:])
                bk_sb = const.tile([P, 2], F32)
                nc.sync.dma_start(bk_sb, bk_d[:])
            if with_bv:
                bv_full = const.tile([P, HGD], F32)
                nc.sync.dma_start(bv_full, bvb_d[:].to_broadcast([P, HGD]))

            # ---- projections (emitted per q-group, interleaved with the
            # ACT-bound attention stream so the PE stays dense) ----
            qT_f32 = const.tile([P, 2, S], F32)
            kT_sb = const.tile([P, 2, S], F32)
            qT_sb = const.tile([P, 2, S], MDT, name='qT_cast') if use_bf16 else qT_f32
            va_sb = const.tile([P, NKC, NH * VW], MDT)
            va_resh = va_sb.rearrange("p s (h c) -> p s h c", c=VW)
            nc.vector.memset(va_resh[:, :, :, HD], 1.0)

            def proj(ns):
                """q/k/v projections for sequence slice ns (one q-group)."""
                sl = slice(ns * QG, (ns + 1) * QG)
                for w_sb, t_sb, b_sb in (
                    (wq_sb, qT_f32, "q"),
                    (wk_sb, kT_sb, "k"),
                ):
                    for mc in range(2):
                        ps = ppool.tile([P, QG], F32, tag="pp")
                        for ki in range(KO):
                            nc.tensor.matmul(
                                ps,
                                lhsT=w_sb[:, ki, mc * P:(mc + 1) * P],
                                rhs=xT_sb[:, ki, sl],
                                start=(ki == 0),
                                stop=(ki == KO - 1),
                            )
                        if with_bqk:
                            bias = (bq_sb if b_sb == "q" else bk_sb)[:, mc:mc + 1]
                            nc.scalar.activation(
                                t_sb[:, mc, sl], ps,
                                mybir.ActivationFunctionType.Identity,
                                bias=bias,
                            )
                        else:
                            nc.scalar.copy(t_sb[:, mc, sl], ps)
                # fold a (and 1/sqrt(HD)) into q^T on the idle GPSIMD
                for mc in range(2):
                    nc.gpsimd.tensor_tensor(
                        qT_sb[:, mc, sl], qT_f32[:, mc, sl],
                        avec_full[:, sl], mybir.AluOpType.mult,
                    )
                for sc in range(4 * ns, 4 * ns + 4):
                    ps = ppool.tile([P, HGD], F32, tag="pp")
                    for ki in range(KO):
                        nc.tensor.matmul(
                            ps,
                            lhsT=xT_sb[:, ki, sc * P:(sc + 1) * P],
                            rhs=wv_sb[:, ki, :],
                            start=(ki == 0),
                            stop=(ki == KO - 1),
                        )
                    for h in range(NH):
                        dst = va_sb[:, sc, h * VW:h * VW + HD]
                        src = ps[:, h * HD:(h + 1) * HD]
                        if with_bv:
                            nc.vector.tensor_tensor(
                                dst, src, bv_full[:, h * HD:(h + 1) * HD],
                                mybir.AluOpType.add,
                            )
                        else:
                            nc.vector.tensor_copy(dst, src)

            # ---- attention + out-projection: software-pipelined q-groups --
            def prep(qg):
                """b-vector broadcast DMA + b-scaled k^T for group qg."""
                klo = kc_lo[qg] * KC
                khi = (qg + 1) * QG
                kw = khi - klo
                bvf = bvqp.tile([P, wmax], F32, tag="bvf")
                nc.sync.dma_start(
                    bvf[:, :kw],
                    bv_d[:][qg:qg + 1, klo:khi].to_broadcast([P, kw]),
                )
                kts = ktsp.tile([P, 2, wmax], MDT, tag="kts")
                for mc in range(2):
                    nc.gpsimd.tensor_tensor(
                        kts[:, mc, :kw], kT_sb[:, mc, klo:khi], bvf[:, :kw],
                        mybir.AluOpType.mult,
                    )
                return kts

            def attn(qg, kts):
                """score/exp/PV chains for all heads; returns ctx psums+dens."""
                klo = kc_lo[qg] * KC
                ctxps = [None] * NH
                dens = densp.tile([NH, QG], F32, tag="dens")
                kcs = list(range(kc_lo[qg], 4 * (qg + 1)))
                for hp in range(2):
                    # two heads of one 128-row kT chunk run CONCURRENTLY on
                    # the PE via row-tiling (array rows 0-63 / 64-127), and
                    # share one 2-bank score tile + one wide exp
                    h0, h1 = 2 * hp, 2 * hp + 1
                    cps_pair = []
                    for h in (h0, h1):
                        cps = cpool.tile([VW, QG], F32, tag="ctx")
                        cps_pair.append(cps)
                    for kc in kcs:
                        q_off = max(0, KC * (kc - 4 * qg))
                        co = kc * KC - klo
                        sp2 = spool.tile([P, 2, QG], F32, tag="spsum")
                        for j, h in enumerate((h0, h1)):
                            pb = (h % 2) * HD
                            nc.tensor.matmul(
                                sp2[:, j, q_off:],
                                lhsT=kts[pb:pb + HD, hp, co:co + KC],
                                rhs=qT_sb[pb:pb + HD, hp,
                                          qg * QG + q_off:(qg + 1) * QG],
                                start=True,
                                stop=True,
                            )
                        if kc >= 4 * qg:  # diagonal: mask both heads' bands
                            band = bass.AP(
                                tensor=sp2.tensor, offset=sp2.offset + q_off,
                                ap=[list(sp2.ap[0]), [QG, 2], [1, KC]],
                            )
                            nc.vector.tensor_tensor(
                                band, band, bm_sb[:, None, :].to_broadcast(
                                    [P, 2, KC]),
                                mybir.AluOpType.add,
                            )
                        pt = ptp.tile([P, 2, QG], MDT, tag="pt")
                        nc.scalar.activation(
                            pt[:, :, q_off:], sp2[:, :, q_off:],
                            mybir.ActivationFunctionType.Exp,
                        )
                        for j, h in enumerate((h0, h1)):
                            nc.tensor.matmul(
                                cps_pair[j][:, q_off:],
                                lhsT=va_sb[:, kc, h * VW:(h + 1) * VW],
                                rhs=pt[:, j, q_off:],
                                start=(kc == kcs[0]),
                                stop=(kc == kcs[-1]),
                            )
                    for j, h in enumerate((h0, h1)):
                        cps = cps_pair[j]
                        # denominator (PSUM partition 64) -> SBUF -> row h
                        d64 = densp.tile([HD + 1, QG], F32, tag="d64")
                        nc.vector.tensor_copy(d64[HD:HD + 1, :],
                                              cps[HD:HD + 1, :])
                        nc.sync.dma_start(dens[h:h + 1, :], d64[HD:HD + 1, :])
                        # undivided ctx to SBUF, freeing the accumulation bank
                        cxf = ctxp.tile([HD, QG], F32, tag="cxf")
                        nc.scalar.copy(cxf, cps[:HD, :])
                        ctxps[h] = cxf
                # add the distant-past count to the denominators
                nc.vector.tensor_scalar_add(dens, dens, cnt_sb[:, qg:qg + 1])
                return ctxps, dens

            def tail(qg, ctxps, dens):
                """reciprocal (via PE transpose + 128-lane DVE), divide ctx,
                out-project, store."""
                dtp = ppool.tile([P, NH * (QG // P)], F32, tag="pp")
                for ss in range(QG // P):
                    nc.tensor.matmul(
                        dtp[:, ss * NH:(ss + 1) * NH],
                        lhsT=dens[:, ss * P:(ss + 1) * P],
                        rhs=eye4_sb,
                        start=True,
                        stop=True,
                    )
                rct = densp.tile([P, NH * (QG // P)], F32, tag="rct")
                nc.vector.reciprocal(rct, dtp)
                rps = ppool.tile([NH, QG], F32, tag="pp")
                for ss in range(QG // P):
                    nc.tensor.matmul(
                        rps[:, ss * P:(ss + 1) * P],
                        lhsT=rct[:, ss * NH:(ss + 1) * NH],
                        rhs=eye128_sb,
                        start=True,
                        stop=True,
                    )
                rec = densp.tile([NH, QG], F32, tag="rec")
                nc.vector.tensor_copy(rec, rps)
                # divide each head, and as soon as a head PAIR is divided,
                # repack it onto one full 128-partition tile (DMA is the only
                # cross-partition mover) so the out-projection runs at K=128
                # instead of two half-array K=64 matmuls
                ctx_sb = []
                pairs = []
                for h in range(NH):
                    recl = densp.tile([1, QG], F32, tag="recl")
                    nc.sync.dma_start(recl, rec[h:h + 1, :])
                    bcs = densp.tile([HD, QG], F32, tag="bcs")
                    nc.gpsimd.partition_broadcast(bcs, recl)
                    csb = ctxp.tile([HD, QG], MDT, tag="ctxsb")
                    ctx_sb.append(csb)
                    # ctx = (near_ctx + distant_prefix) * (1/den)
                    nc.vector.scalar_tensor_tensor(
                        csb, ctxps[h],
                        pft_sb[:HD, qg * NH + h:qg * NH + h + 1],
                        bcs,
                        mybir.AluOpType.add,
                        mybir.AluOpType.mult,
                    )
                    if h % 2 == 1:
                        cp2 = ctxp.tile([P, QG], MDT, tag="cpair")
                        nc.sync.dma_start(cp2[0:HD, :], ctx_sb[h - 1])
                        nc.sync.dma_start(cp2[HD:P, :], ctx_sb[h])
                        pairs.append(cp2)
                for ss in range(QG // P):
                    ops = ppool.tile([P, DM], F32, tag="pp")
                    for hp in range(2):
                        nc.tensor.matmul(
                            ops,
                            lhsT=pairs[hp][:, ss * P:(ss + 1) * P],
                            rhs=wo_sb[:, hp, :],
                            start=(hp == 0),
                            stop=(hp == 1),
                        )
                    osb = ptp.tile([P, DM], F32, tag="osb")
                    nc.vector.tensor_copy(osb, ops)
                    nc.sync.dma_start(
                        out_d[:][qg * QG + ss * P:qg * QG + (ss + 1) * P, :],
                        osb,
                    )

            # two-group lookahead: projections + scaled-k prefetch run well
            # ahead of the attention group that consumes them
            proj(0)
            ktss = [prep(0)]
            proj(1)
            ktss.append(prep(1))
            pending = None
            for qg in range(NQG):
                ctxps, dens = attn(qg, ktss[qg])
                if qg + 2 < NQG:
                    proj(qg + 2)
                    ktss.append(prep(qg + 2))
                if pending is not None:
                    tail(*pending)
                pending = (qg, ctxps, dens)
            tail(*pending)

    nc.finalize()
    return nc


# --------------------------------------------------------------------------
# host wrapper
# --------------------------------------------------------------------------

def _is_tril(mask: np.ndarray) -> bool:
    tril = np.tril(np.ones((S, S), dtype=mask.dtype))
    return all(np.array_equal(mask[b], tril) for b in range(mask.shape[0]))


def _prep_core_inputs(x, days, Wq, bq, Wk, bk, Wv, bv, Wo, rate,
                      use_bf16):
    """Per-core in_maps plus static loop bounds (shared across cores)."""
    t = days.astype(np.float64)  # [B, S]
    # distance beyond which exp(s * decay) == 1.0f exactly: need
    # |s| * exp(-rate*d) < 2^-25 with a generous |s| <= 150 bound.
    d_cut = (np.log(150.0) + 25.5 * np.log(2.0)) / rate
    # static near-window bounds (min over batches so one program fits all)
    kc_lo = []
    for qg in range(NQG):
        lo = NKC
        for b in range(B):
            tq = t[b, qg * QG]
            c = 0
            while c < 4 * qg and t[b, c * KC + KC - 1] < tq - d_cut:
                c += 1
            lo = min(lo, c)
        kc_lo.append(lo)
    kc_lo = tuple(kc_lo)
    wmax = max((qg + 1) * QG - kc_lo[qg] * KC for qg in range(NQG))
    wmax = ((wmax + P - 1) // P) * P

    # per-batch decay factor vectors (f64 for exactness, then f32)
    scale = 1.0 / np.sqrt(HD)
    t0 = np.stack([(t[:, qg * QG] + t[:, qg * QG + QG - 1]) * 0.5
                   for qg in range(NQG)], axis=1)  # [B, NQG]
    avec = np.zeros((B, 1, S), np.float32)
    bvec = np.zeros((B, NQG, S), np.float32)
    for b in range(B):
        for qg in range(NQG):
            sl = slice(qg * QG, (qg + 1) * QG)
            avec[b, 0, sl] = (np.exp(-rate * (t[b, sl] - t0[b, qg])) * scale
                              ).astype(np.float32)
            hi = (qg + 1) * QG
            bvec[b, qg, :hi] = (np.exp(rate * (t[b, :hi] - t0[b, qg]))
                                ).astype(np.float32)
    assert np.all(np.isfinite(avec)) and np.all(np.isfinite(bvec)), \
        "decay factor overflow; q-group span too large for fast path"

    # band mask: keep (0.0) iff q_local >= k_local else -1e30
    kl = np.arange(P)[:, None]
    ql = np.arange(P)[None, :]
    bandm = np.where(ql >= kl, 0.0, NEG).astype(np.float32)

    with_bqk = bool(np.any(bq != 0) or np.any(bk != 0))
    with_bv = bool(np.any(bv != 0))

    in_maps = []
    for c in range(NCORES):
        b, hg = divmod(c, NHG)
        cols = slice(hg * HGD, (hg + 1) * HGD)
        # prefix V sums for the distant rank-1 update: [HD, NQG*NH] (hd-major)
        prefv = np.zeros((HD, NQG * NH), np.float32)
        cnt = np.zeros((NH, NQG), np.float32)
        for qg in range(NQG):
            n = kc_lo[qg] * KC
            cnt[:, qg] = float(n)
            if n > 0:
                xs = x[b, :n].astype(np.float64).sum(axis=0)  # [DM]
                vs = xs @ Wv[cols, :].astype(np.float64).T \
                    + n * bv[cols].astype(np.float64)
                for h in range(NH):
                    prefv[:, qg * NH + h] = \
                        vs[h * HD:(h + 1) * HD].astype(np.float32)
        mdt = np.dtype(ml_dtypes.bfloat16) if use_bf16 else np.float32
        m = {
            "xT": np.ascontiguousarray(x[b].T).astype(mdt),
            "wqT": np.ascontiguousarray(Wq[cols, :].T).astype(mdt),
            "wkT": np.ascontiguousarray(Wk[cols, :].T).astype(mdt),
            "wvT": np.ascontiguousarray(Wv[cols, :].T).astype(mdt),
            "woT": np.ascontiguousarray(Wo[:, cols].T).astype(mdt),
            "avec": avec[b],
            "bvec": bvec[b],
            "prefv": prefv,
            "cnt": cnt,
            "bandm": bandm,
        }
        if with_bqk:
            m["bq"] = np.ascontiguousarray(
                bq[cols].reshape(2, P).T).astype(np.float32)
            m["bk"] = np.ascontiguousarray(
                bk[cols].reshape(2, P).T).astype(np.float32)
        if with_bv:
            m["bvb"] = bv[cols].reshape(1, HGD).astype(np.float32)
        in_maps.append(m)
    return in_maps, kc_lo, wmax, with_bqk, with_bv


def _reference_host(x, mask, days_offset, Wq, bq, Wk, bk, Wv, bv, Wo, bo,
                    decay_rate):
    """Emergency numpy fallback for inputs outside the fast path."""
    b, s, _ = x.shape
    out = np.empty((b, s, DM), np.float32)
    for bi in range(b):
        q = (x[bi] @ Wq.T + bq).reshape(s, H, HD).transpose(1, 0, 2)
        k = (x[bi] @ Wk.T + bk).reshape(s, H, HD).transpose(1, 0, 2)
        v = (x[bi] @ Wv.T + bv).reshape(s, H, HD).transpose(1, 0, 2)
        dist = np.abs(days_offset[bi][:, None] - days_offset[bi][None, :])
        decay = np.exp(-decay_rate * dist).astype(np.float32)
        ctx = np.empty((H, s, HD), np.float32)
        for h in range(H):
            sc = (q[h] @ k[h].T) / np.sqrt(HD) * decay
            sc = np.where(mask[bi] == 0, -np.inf, sc)
            sc = sc - sc.max(axis=-1, keepdims=True)
            e = np.exp(sc)
            ctx[h] = (e / e.sum(axis=-1, keepdims=True)) @ v[h]
        out[bi] = ctx.transpose(1, 0, 2).reshape(s, DM) @ Wo.T + bo
    return out


def kernel(x, mask, days_offset, Wq, bq, Wk, bk, Wv, bv, Wo, bo, decay_rate,
           _trace=False):
    x = np.asarray(x, np.float32)
    mask = np.asarray(mask)
    days = np.asarray(days_offset, np.float32)
    Wq, bq = np.asarray(Wq, np.float32), np.asarray(bq, np.float32)
    Wk, bk = np.asarray(Wk, np.float32), np.asarray(bk, np.float32)
    Wv, bv = np.asarray(Wv, np.float32), np.asarray(bv, np.float32)
    Wo, bo = np.asarray(Wo, np.float32), np.asarray(bo, np.float32)
    rate = float(np.asarray(decay_rate))

    sorted_ok = bool(np.all(np.diff(days, axis=-1) >= 0))
    if not (sorted_ok and _is_tril(mask)):
        return _reference_host(x, mask, days, Wq, bq, Wk, bk, Wv, bv, Wo, bo,
                               rate)

    use_bf16 = os.environ.get("KERNEL_F32", "") != "1"
    in_maps, kc_lo, wmax, with_bqk, with_bv = _prep_core_inputs(
        x, days, Wq, bq, Wk, bk, Wv, bv, Wo, rate, use_bf16)

    key = (kc_lo, wmax, with_bqk, with_bv, use_bf16)
    if key not in _cache:
        _cache[key] = _build_fast(kc_lo, wmax, with_bqk, with_bv, use_bf16)
    nc = _cache[key]

    res = run_bass_kernel_spmd(nc, in_maps, core_ids=list(range(NCORES)),
                               trace=_trace)
    out = np.empty((B, S, DM), np.float32)
    for b in range(B):
        out[b] = res.results[2 * b]["outp"] + res.results[2 * b + 1]["outp"] + bo
    if _trace:
        return out, res
    return out
